# revision 1
# baseline (speedup 1.0000x reference)
"""Trainium2 Bass kernel for nn_CMEncoder (cross-attention + LayerNorm2d + MLP block).

Strategy (8 NeuronCores, sequence-parallel over the HW=4096 query tokens):
  - Each core owns 512 query tokens; K/V over the full 4096-token context are
    computed redundantly on every core (no collectives needed).
  - Everything stays channel-major on chip ([feature partition, token free]).
  - Scores are computed transposed (S^T[n, q]) so P = exp(S^T) is the moving
    operand of the P@V matmuls (att^T = V^T @ P); the softmax denominator
    comes from a cheap ones-stationary reduction matmul.
  - The attention loop is software-pipelined by one context chunk so the
    softmax-exp (ACT) latency stays off the PE's critical path.
  - Host-side algebraic folds: bk dropped (softmax shift invariance), bv folded
    into the output-projection bias, the 1/sqrt(C) scale folded into the Q
    bias/scale, LayerNorm's affine folded into the MLP's first layer.
  - Matmuls run in bf16 (FWL weight loads hide under the streams).
"""

import math
import numpy as np
import concourse.bacc as bacc
import concourse.mybir as mybir
import concourse.tile as tile
from concourse import bass_utils
from concourse.hw_specs import get_activation_tables

F32 = mybir.dt.float32
F32R = mybir.dt.float32r
BF16 = mybir.dt.bfloat16
AF = mybir.ActivationFunctionType
ALU = mybir.AluOpType

MMDT = F32R      # matmul operand dtype (F32R or BF16)

C = 256          # channels
HW = 4096        # query tokens (64x64)
NCTX = 4096      # context tokens
HID = 512        # mlp hidden
NCORES = 8
QS = HW // NCORES   # 512 queries per core
QH = QS // 2        # 256 queries per half
NBLK = NCTX // 128  # 32 context chunks
EPS = 1e-6


def _build_nc():
    nc = bacc.Bacc("TRN2", target_bir_lowering=False)

    # --- DRAM I/O (weights pre-packed on host: row-chunks side by side) ---
    d_xmm = nc.dram_tensor("x_mm", (128, 2 * QS), MMDT, kind="ExternalInput")
    d_xf = nc.dram_tensor("x_f32", (C, QS), F32, kind="ExternalInput")
    d_y = nc.dram_tensor("y_mm", (128, 2 * NCTX), MMDT, kind="ExternalInput")
    d_wq = nc.dram_tensor("wq_mm", (128, 2 * C), MMDT, kind="ExternalInput")
    d_wk = nc.dram_tensor("wk_mm", (128, 2 * C), MMDT, kind="ExternalInput")
    d_wv = nc.dram_tensor("wv_mm", (128, 2 * C), MMDT, kind="ExternalInput")
    d_wo = nc.dram_tensor("wo_mm", (128, 2 * C), MMDT, kind="ExternalInput")
    d_w1 = nc.dram_tensor("w1_mm", (128, 2 * HID), MMDT, kind="ExternalInput")
    d_w2 = nc.dram_tensor("w2_mm", (128, 4 * C), MMDT, kind="ExternalInput")
    d_bv = nc.dram_tensor("bvec", (C, 3), F32, kind="ExternalInput")   # [bq/16, bo', b2]
    d_b1 = nc.dram_tensor("b1p", (128, 4), F32, kind="ExternalInput")
    d_oc = nc.dram_tensor("ones_c", (128, 2), MMDT, kind="ExternalInput")
    d_or = nc.dram_tensor("ones_r", (1, 128), MMDT, kind="ExternalInput")
    d_out = nc.dram_tensor("out_sh", (C, QS), F32, kind="ExternalOutput")

    tabs = list(get_activation_tables(nc.m.arch).keys())
    LNEXP_SET = tabs.index("natural_log_exp_and_others")

    with tile.TileContext(nc) as tc:
        # Pre-load the exp+ln activation table once so the auto-inserted loads
        # don't ping-pong between exp-only and ln-only sets mid-kernel.
        nc.scalar.add_instruction(mybir.InstLoadActFuncSet(
            name=nc.get_next_instruction_name(), ins=[], outs=[],
            act_func_set_id=LNEXP_SET))

        with (
            tc.tile_pool(name="sb", bufs=1) as sb,
            tc.tile_pool(name="pt_pool", bufs=3) as ptp,
            tc.tile_pool(name="ps", bufs=4, space="PSUM") as ps,
        ):
            # ---------------- input DMAs ----------------
            xmm = sb.tile([128, 2 * QS], MMDT)
            nc.sync.dma_start(xmm, d_xmm[:, :])
            wq_t = sb.tile([128, 2 * C], MMDT)
            nc.sync.dma_start(wq_t, d_wq[:, :])
            yt = [[sb.tile([128, NCTX // 2], MMDT, name=f"y{i}{h}") for h in range(2)]
                  for i in range(2)]
            for h in range(2):
                for i in range(2):
                    nc.sync.dma_start(
                        yt[i][h],
                        d_y[:, i * NCTX + h * (NCTX // 2):
                            i * NCTX + (h + 1) * (NCTX // 2)])

            wk_t = sb.tile([128, 2 * C], MMDT)
            nc.gpsimd.dma_start(wk_t, d_wk[:, :])
            wv_t = sb.tile([128, 2 * C], MMDT)
            nc.gpsimd.dma_start(wv_t, d_wv[:, :])
            bvec = [sb.tile([128, 3], F32, name=f"bvec{i}") for i in range(2)]
            for i in range(2):
                nc.gpsimd.dma_start(bvec[i], d_bv[i * 128:(i + 1) * 128, :])
            ones_c = sb.tile([128, 2], MMDT)
            nc.gpsimd.dma_start(ones_c, d_oc[:, :])
            ones_r = sb.tile([1, 128], MMDT)
            nc.gpsimd.dma_start(ones_r, d_or[:, :])
            wo_t = sb.tile([128, 2 * C], MMDT)
            nc.gpsimd.dma_start(wo_t, d_wo[:, :])
            w1_t = sb.tile([128, 2 * HID], MMDT)
            nc.gpsimd.dma_start(w1_t, d_w1[:, :])
            w2_t = sb.tile([128, 4 * C], MMDT)
            nc.gpsimd.dma_start(w2_t, d_w2[:, :])
            b1p = sb.tile([128, 4], F32)
            nc.gpsimd.dma_start(b1p, d_b1[:, :])
            xf = [sb.tile([128, QS], F32, name=f"xf{i}") for i in range(2)]
            for i in range(2):
                nc.gpsimd.dma_start(xf[i], d_xf[i * 128:(i + 1) * 128, :])

            epsv = sb.tile([1, 1], F32)
            nc.vector.memset(epsv, EPS)
            eps2v = sb.tile([1, 1], F32)
            nc.vector.memset(eps2v, float(C) * float(C) * EPS)
            lnCv = sb.tile([1, 1], F32)
            nc.vector.memset(lnCv, math.log(float(C)))

            def wsl(t, cc, cb, w=128):
                # packed weight tile slice: row-chunk cc, col-chunk cb
                return t[:, cc * (t.shape[1] // 2) + cb * w:
                         cc * (t.shape[1] // 2) + (cb + 1) * w]

            # ---------------- Q' = (x^T Wq^T + bq)/16, channel-major ----------------
            qp = [sb.tile([128, QS], MMDT, name=f"qp{i}") for i in range(2)]
            for cb in range(2):
                qps = ps.tile([128, 512], F32, tag="work", name=f"qps{cb}")
                nc.tensor.matmul(qps, wsl(wq_t, 0, cb), xmm[:, 0:QS],
                                 start=True, stop=False)
                nc.tensor.matmul(qps, wsl(wq_t, 1, cb), xmm[:, QS:2 * QS],
                                 start=False, stop=True)
                nc.scalar.activation(qp[cb], qps, AF.Identity,
                                     bias=bvec[cb][:, 0:1], scale=1.0 / 16.0)

            # ---------------- K^T and V (token-major) ----------------
            kt = [sb.tile([128, NCTX], MMDT, name=f"kt{i}") for i in range(2)]
            v_all = sb.tile([128, NBLK * 256], MMDT)
            for nb in range(8):
                h = nb // 4
                col = (nb % 4) * 512
                for cb in range(2):
                    kps = ps.tile([128, 512], F32, tag="work", name=f"kps{cb}_{nb}")
                    nc.tensor.matmul(kps, wsl(wk_t, 0, cb),
                                     yt[0][h][:, col:col + 512], start=True, stop=False)
                    nc.tensor.matmul(kps, wsl(wk_t, 1, cb),
                                     yt[1][h][:, col:col + 512], start=False, stop=True)
                    nc.scalar.copy(kt[cb][:, nb * 512:(nb + 1) * 512], kps)
                for p2 in range(2):
                    vps = ps.tile([128, 512], F32, tag="work", name=f"vps{nb}_{p2}")
                    for k in range(2):
                        ci = nb * 4 + p2 * 2 + k
                        c0 = (ci * 128) % 2048
                        nc.tensor.matmul(vps[:, k * 256:(k + 1) * 256],
                                         yt[0][h][:, c0:c0 + 128],
                                         wv_t[:, 0:256], start=True, stop=False)
                        nc.tensor.matmul(vps[:, k * 256:(k + 1) * 256],
                                         yt[1][h][:, c0:c0 + 128],
                                         wv_t[:, 256:512], start=False, stop=True)
                    ci0 = nb * 4 + p2 * 2
                    nc.vector.tensor_copy(v_all[:, ci0 * 256:(ci0 + 2) * 256], vps)

            # ---------------- attention state ----------------
            attps = [ps.tile([128, QS], F32, tag=f"attps{j}", bufs=1,
                             name=f"attps{j}") for j in range(2)]
            csum = ps.tile([2, QS], F32, tag="csum", bufs=1)

            # full-width SBUF tensors, written per half
            attnT = [sb.tile([128, QS], MMDT, name=f"attnT{i}") for i in range(2)]
            zs = [sb.tile([128, QS], MMDT, name=f"zs{i}") for i in range(2)]
            zsq = [sb.tile([128, QS], MMDT, name=f"zsq{i}") for i in range(2)]
            zln = [sb.tile([128, QS], MMDT, name=f"zln{i}") for i in range(2)]
            hs = [sb.tile([128, QS], MMDT, name=f"hs{i}") for i in range(4)]
            att_s = [sb.tile([128, QS], MMDT, name=f"att_s{i}") for i in range(2)]
            ot = [sb.tile([128, QS], F32, name=f"ot{i}") for i in range(2)]
            rstd = sb.tile([1, QS], MMDT)
            nmrs = sb.tile([1, QS], MMDT)
            lncs = sb.tile([1, QS], F32)
            rr = sb.tile([1, QS], MMDT)
            neg_mean = sb.tile([1, QS], F32)
            m2 = sb.tile([1, QS], F32)
            var = sb.tile([1, QS], F32)
            lnv = sb.tile([1, QS], F32)

            def attn_score(i):
                """S^T and exp for context chunk i"""
                sps = ps.tile([128, QS], F32, tag="work", name=f"sps{i}")
                nc.tensor.matmul(sps, kt[0][:, i * 128:(i + 1) * 128], qp[0],
                                 start=True, stop=False)
                nc.tensor.matmul(sps, kt[1][:, i * 128:(i + 1) * 128], qp[1],
                                 start=False, stop=True)
                pt = ptp.tile([128, QS], MMDT, tag="pt", name=f"pt{i}")
                nc.scalar.activation(pt, sps, AF.Exp)
                return pt

            def attn_accum(i, pt):
                """P@V and colsum accumulation for chunk i"""
                first, last = (i == 0), (i == NBLK - 1)
                for cb in range(2):
                    nc.tensor.matmul(
                        attps[cb],
                        v_all[:, i * 256 + cb * 128:i * 256 + (cb + 1) * 128],
                        pt, start=first, stop=last)
                nc.tensor.matmul(csum, ones_c, pt, start=first, stop=last)

            # ---- attention, software-pipelined by one chunk so the exp
            # ---- latency sits off the PE's static instruction order ----
            prev = attn_score(0)
            for i in range(1, NBLK):
                cur = attn_score(i)
                attn_accum(i - 1, prev)
                prev = cur
            attn_accum(NBLK - 1, prev)


            # softmax normalize: 1/colsum via exp(-ln(x)) on ACT
            nc.scalar.activation(lncs, csum[0:1, :], AF.Ln)
            nc.scalar.activation(rr, lncs, AF.Exp, scale=-1.0)
            rb = ps.tile([128, QS], F32, tag="work", name="rb")
            nc.tensor.matmul(rb, ones_r, rr, start=True, stop=True)
            for cb in range(2):
                nc.vector.tensor_copy(att_s[cb], attps[cb])
                nc.vector.tensor_mul(attnT[cb], att_s[cb], rb)

            # z = Wo @ attnT + bo', LayerNorm stats
            for cb in range(2):
                zps = ps.tile([128, QS], F32, tag="work", name=f"zps{cb}")
                nc.tensor.matmul(zps, wsl(wo_t, 0, cb), attnT[0], start=True, stop=False)
                nc.tensor.matmul(zps, wsl(wo_t, 1, cb), attnT[1], start=False, stop=True)
                nc.scalar.activation(zs[cb], zps, AF.Identity, bias=bvec[cb][:, 1:2])
                nc.vector.tensor_mul(zsq[cb], zs[cb], zs[cb])

            szp = ps.tile([2, QS], F32, tag="work", name="szp")
            nc.tensor.matmul(szp, ones_c, zs[0], start=True, stop=False)
            nc.tensor.matmul(szp, ones_c, zs[1], start=False, stop=True)
            sqp = ps.tile([2, QS], F32, tag="work", name="sqp")
            nc.tensor.matmul(sqp, ones_c, zsq[0], start=True, stop=False)
            nc.tensor.matmul(sqp, ones_c, zsq[1], start=False, stop=True)

            s2 = sb.tile([1, QS], F32)
            nc.scalar.square(s2, szp[0:1, :])
            nc.vector.scalar_tensor_tensor(var, sqp[0:1, :], float(C), s2,
                                           op0=ALU.mult, op1=ALU.subtract)
            nc.scalar.activation(lnv, var, AF.Ln, bias=eps2v)
            nc.scalar.activation(rstd, lnv, AF.Exp, scale=-0.5, bias=lnCv)
            nc.vector.tensor_scalar_mul(neg_mean, szp[0:1, :], -1.0 / C)
            nc.vector.tensor_mul(nmrs, neg_mean, rstd)

            rstd_b = ps.tile([128, QS], F32, tag="work", name="rstd_b")
            nc.tensor.matmul(rstd_b, ones_r, rstd, start=True, stop=True)
            nmrs_b = ps.tile([128, QS], F32, tag="work", name="nmrs_b")
            nc.tensor.matmul(nmrs_b, ones_r, nmrs, start=True, stop=True)

            for cb in range(2):
                zt = sb.tile([128, QS], MMDT, name=f"zt{cb}")
                nc.vector.tensor_mul(zt, zs[cb], rstd_b)
                nc.vector.tensor_add(zln[cb], zt, nmrs_b)

            # MLP + residual
            for hb in range(4):
                hps = ps.tile([128, QS], F32, tag="work", name=f"hps{hb}")
                nc.tensor.matmul(hps, wsl(w1_t, 0, hb), zln[0], start=True, stop=False)
                nc.tensor.matmul(hps, wsl(w1_t, 1, hb), zln[1], start=False, stop=True)
                nc.scalar.activation(hs[hb], hps, AF.Gelu, bias=b1p[:, hb:hb + 1])

            for cb in range(2):
                tps2 = ps.tile([128, QS], F32, tag="work", name=f"tps2{cb}")
                for hb in range(4):
                    nc.tensor.matmul(
                        tps2, w2_t[:, hb * 256 + cb * 128:hb * 256 + (cb + 1) * 128],
                        hs[hb], start=(hb == 0), stop=(hb == 3))
                nc.vector.scalar_tensor_tensor(ot[cb], tps2, bvec[cb][:, 2:3], xf[cb],
                                               op0=ALU.add, op1=ALU.add)
                nc.sync.dma_start(d_out[cb * 128:(cb + 1) * 128, :], ot[cb])

    nc.compile()
    return nc


_NC = None


def _get_nc():
    global _NC
    if _NC is None:
        _NC = _build_nc()
    return _NC


def _pack_rows(a, nchunk):
    """(nchunk*128, W) -> (128, nchunk*W) with row-chunks side by side."""
    w = a.shape[1]
    out = np.empty((128, nchunk * w), a.dtype)
    for i in range(nchunk):
        out[:, i * w:(i + 1) * w] = a[i * 128:(i + 1) * 128, :]
    return out


def prep_in_maps(x, y, Wq, bq, Wk, bk, Wv, bv, Wo, bo, ln_w, ln_b, W1, b1, W2, b2):
    f = lambda a: np.asarray(a, dtype=np.float32)
    x, y = f(x), f(y)
    Wq, bq, Wk, Wv, bv, Wo, bo = f(Wq), f(bq), f(Wk), f(Wv), f(bv), f(Wo), f(bo)
    ln_w, ln_b, W1, b1, W2, b2 = f(ln_w), f(ln_b), f(W1), f(b1), f(W2), f(b2)

    mmnp = mybir.dt.np(MMDT)
    g = lambda a: np.ascontiguousarray(a).astype(mmnp)

    x_cm = np.ascontiguousarray(x.reshape(C, HW))
    y_cm = np.ascontiguousarray(y.reshape(C, NCTX))

    # host-side algebraic folds
    bo_p = (Wo.astype(np.float64) @ bv.astype(np.float64) + bo).astype(np.float32)
    b1_p = (W1.astype(np.float64) @ ln_b.astype(np.float64) + b1).astype(np.float32)
    W1p = (W1 * ln_w[None, :]).astype(np.float32)

    bvec = np.stack([bq / 16.0, bo_p, b2], axis=1).astype(np.float32)  # (256,3)

    common = {
        "y_mm": g(_pack_rows(y_cm, 2)),
        "wq_mm": g(_pack_rows(Wq.T, 2)),
        "wk_mm": g(_pack_rows(Wk.T, 2)),
        "wv_mm": g(_pack_rows(Wv.T, 2)),
        "wo_mm": g(_pack_rows(Wo.T, 2)),
        "w1_mm": g(_pack_rows(W1p.T, 2)),
        "w2_mm": g(_pack_rows(W2.T, 4)),
        "bvec": bvec,
        "b1p": np.ascontiguousarray(b1_p.reshape(4, 128).T),
        "ones_c": np.ones((128, 2), mmnp),
        "ones_r": np.ones((1, 128), mmnp),
    }
    in_maps = []
    for i in range(NCORES):
        m = dict(common)
        xs = np.ascontiguousarray(x_cm[:, i * QS:(i + 1) * QS])
        m["x_f32"] = xs
        m["x_mm"] = g(_pack_rows(xs, 2))
        in_maps.append(m)
    return in_maps


def kernel(**inputs):
    in_maps = prep_in_maps(**inputs)
    nc = _get_nc()
    res = bass_utils.run_bass_kernel_spmd(nc, in_maps, core_ids=list(range(NCORES)))
    t = np.concatenate([res.results[i]["out_sh"] for i in range(NCORES)], axis=1)
    return t.reshape(1, C, 64, 64)



# revision 2
# speedup vs baseline: 1.0267x; 1.0267x over previous
"""Trainium2 Bass kernel for nn_CMEncoder (cross-attention + LayerNorm2d + MLP block).

Strategy (8 NeuronCores, sequence-parallel over the HW=4096 query tokens):
  - Each core owns 512 query tokens; K/V over the full 4096-token context are
    computed redundantly on every core (no collectives needed).
  - Everything stays channel-major on chip ([feature partition, token free]).
  - Scores are computed transposed (S^T[n, q]) so P = exp(S^T) is the moving
    operand of the P@V matmuls (att^T = V^T @ P); the softmax denominator
    comes from a cheap ones-stationary reduction matmul.
  - All matmuls run in bf16 (2.4 GHz streaming + FWL weight loads, vs 1.2 GHz
    and no FWL for fp32) with fp32 PSUM accumulation.
  - Context chunks are processed in PAIRS: two 512-col matmul outputs land in
    adjacent PSUM banks of one [128,1024] tile so a single ACT instruction
    does the exp (amortizes the ~352-cycle ACT fixed overhead).
  - PSUM->SBUF evacuations are split between the scalar and vector engines.
  - Host-side algebraic folds: bk dropped (softmax shift invariance), bv folded
    into the output-projection bias, the 1/sqrt(C) scale folded into the Q
    bias/scale, LayerNorm's affine folded into the MLP's first layer.
  - The act-table set switches exp/ln -> gelu right after the last ln/exp use,
    hidden behind the LN broadcast/apply chain.
"""

import math
import numpy as np
import concourse.bacc as bacc
import concourse.mybir as mybir
import concourse.tile as tile
from concourse import bass_utils
from concourse.hw_specs import get_activation_tables

F32 = mybir.dt.float32
BF16 = mybir.dt.bfloat16
AF = mybir.ActivationFunctionType
ALU = mybir.AluOpType

MMDT = BF16      # matmul operand dtype

C = 256          # channels
HW = 4096        # query tokens (64x64)
NCTX = 4096      # context tokens
HID = 512        # mlp hidden
NCORES = 8
QS = HW // NCORES   # 512 queries per core
NBLK = NCTX // 128  # 32 context chunks
NPAIR = NBLK // 2   # 16 chunk pairs
EPS = 1e-6


def _build_nc():
    nc = bacc.Bacc("TRN2", target_bir_lowering=False)

    # --- DRAM I/O (weights pre-packed on host: row-chunks side by side) ---
    d_xmm = nc.dram_tensor("x_mm", (128, 2 * QS), MMDT, kind="ExternalInput")
    d_xf = nc.dram_tensor("x_f32", (C, QS), F32, kind="ExternalInput")
    d_y = nc.dram_tensor("y_mm", (128, 2 * NCTX), MMDT, kind="ExternalInput")
    d_wq = nc.dram_tensor("wq_mm", (128, 2 * C), MMDT, kind="ExternalInput")
    d_wk = nc.dram_tensor("wk_mm", (128, 2 * C), MMDT, kind="ExternalInput")
    d_wv = nc.dram_tensor("wv_mm", (128, 2 * C), MMDT, kind="ExternalInput")
    d_wo = nc.dram_tensor("wo_mm", (128, 2 * C), MMDT, kind="ExternalInput")
    d_w1 = nc.dram_tensor("w1_mm", (128, 2 * HID), MMDT, kind="ExternalInput")
    d_w2 = nc.dram_tensor("w2_mm", (128, 4 * C), MMDT, kind="ExternalInput")
    d_bv = nc.dram_tensor("bvec", (C, 3), F32, kind="ExternalInput")   # [bq/16, bo', b2]
    d_b1 = nc.dram_tensor("b1p", (128, 4), F32, kind="ExternalInput")
    d_oc = nc.dram_tensor("ones_c", (128, 2), MMDT, kind="ExternalInput")
    d_or = nc.dram_tensor("ones_r", (1, 128), MMDT, kind="ExternalInput")
    d_out = nc.dram_tensor("out_sh", (C, QS), F32, kind="ExternalOutput")

    tabs = list(get_activation_tables(nc.m.arch).keys())
    LNEXP_SET = tabs.index("natural_log_exp_and_others")
    GELU_SET = tabs.index("gelu_and_others")

    def load_table(set_id):
        nc.scalar.add_instruction(mybir.InstLoadActFuncSet(
            name=nc.get_next_instruction_name(), ins=[], outs=[],
            act_func_set_id=set_id))

    with tile.TileContext(nc) as tc:
        load_table(LNEXP_SET)

        with (
            tc.tile_pool(name="sb", bufs=1) as sb,
            tc.tile_pool(name="pt_pool", bufs=3) as ptp,
            tc.tile_pool(name="ps", bufs=2, space="PSUM") as ps,
        ):
            # ---------------- input DMAs ----------------
            # sync queue: the tensors the PE needs first, in need-order.
            wq_t = sb.tile([128, 2 * C], MMDT)
            nc.sync.dma_start(wq_t, d_wq[:, :])
            xmm = sb.tile([128, 2 * QS], MMDT)
            nc.sync.dma_start(xmm, d_xmm[:, :])
            wk_t = sb.tile([128, 2 * C], MMDT)
            nc.sync.dma_start(wk_t, d_wk[:, :])
            # y in ctx-quarters, both channel-halves of a quarter back to back
            yq = [[None] * 4 for _ in range(2)]
            for q in range(4):
                for i in range(2):
                    yq[i][q] = sb.tile([128, 1024], MMDT, name=f"y{i}{q}")
                    nc.sync.dma_start(
                        yq[i][q], d_y[:, i * NCTX + q * 1024:i * NCTX + (q + 1) * 1024])

            # gpsimd queue: everything needed later.
            wv_t = sb.tile([128, 2 * C], MMDT)
            nc.gpsimd.dma_start(wv_t, d_wv[:, :])
            ones_c = sb.tile([128, 2], MMDT)
            nc.gpsimd.dma_start(ones_c, d_oc[:, :])
            ones_r = sb.tile([1, 128], MMDT)
            nc.gpsimd.dma_start(ones_r, d_or[:, :])
            bvec = [sb.tile([128, 3], F32, name=f"bvec{i}") for i in range(2)]
            for i in range(2):
                nc.gpsimd.dma_start(bvec[i], d_bv[i * 128:(i + 1) * 128, :])
            wo_t = sb.tile([128, 2 * C], MMDT)
            nc.gpsimd.dma_start(wo_t, d_wo[:, :])
            w1_t = sb.tile([128, 2 * HID], MMDT)
            nc.gpsimd.dma_start(w1_t, d_w1[:, :])
            w2_t = sb.tile([128, 4 * C], MMDT)
            nc.gpsimd.dma_start(w2_t, d_w2[:, :])
            b1p = sb.tile([128, 4], F32)
            nc.gpsimd.dma_start(b1p, d_b1[:, :])
            xf = [sb.tile([128, QS], F32, name=f"xf{i}") for i in range(2)]
            for i in range(2):
                nc.gpsimd.dma_start(xf[i], d_xf[i * 128:(i + 1) * 128, :])

            eps2v = sb.tile([1, 1], F32)
            nc.vector.memset(eps2v, float(C) * float(C) * EPS)
            lnCv = sb.tile([1, 1], F32)
            nc.vector.memset(lnCv, math.log(float(C)))

            def wsl(t, cc, cb, w=128):
                # packed weight tile slice: row-chunk cc, col-chunk cb
                return t[:, cc * (t.shape[1] // 2) + cb * w:
                         cc * (t.shape[1] // 2) + (cb + 1) * w]

            def yslice(i, c0, w):
                # y channel-half i, ctx cols [c0, c0+w) (must stay in a quarter)
                q, o = c0 // 1024, c0 % 1024
                return yq[i][q][:, o:o + w]

            # ---------------- Q' = (x^T Wq^T + bq)/16, channel-major ----------------
            qp2 = sb.tile([128, 2 * QS], MMDT)
            qps = ps.tile([128, 1024], F32, tag="w", name="qps")
            for cb in range(2):
                nc.tensor.matmul(qps[:, cb * 512:(cb + 1) * 512],
                                 wsl(wq_t, 0, cb), xmm[:, 0:QS],
                                 start=True, stop=False)
                nc.tensor.matmul(qps[:, cb * 512:(cb + 1) * 512],
                                 wsl(wq_t, 1, cb), xmm[:, QS:2 * QS],
                                 start=False, stop=True)
            for cb in range(2):
                nc.scalar.activation(qp2[:, cb * 512:(cb + 1) * 512],
                                     qps[:, cb * 512:(cb + 1) * 512], AF.Identity,
                                     bias=bvec[cb][:, 0:1], scale=1.0 / 16.0)

            # ---------------- K^T (channel-major) ----------------
            kt = [sb.tile([128, NCTX], MMDT, name=f"kt{i}") for i in range(2)]
            ev = 0  # evacuation engine round-robin
            for qq in range(4):
                for cb in range(2):
                    kps = ps.tile([128, 1024], F32, tag="w", name=f"kps{qq}{cb}")
                    for h in range(2):
                        c0 = qq * 1024 + h * 512
                        nc.tensor.matmul(kps[:, h * 512:(h + 1) * 512],
                                         wsl(wk_t, 0, cb), yslice(0, c0, 512),
                                         start=True, stop=False)
                        nc.tensor.matmul(kps[:, h * 512:(h + 1) * 512],
                                         wsl(wk_t, 1, cb), yslice(1, c0, 512),
                                         start=False, stop=True)
                    dst = kt[cb][:, qq * 1024:(qq + 1) * 1024]
                    if ev % 2 == 0:
                        nc.scalar.copy(dst, kps)
                    else:
                        nc.vector.tensor_copy(dst, kps)
                    ev += 1

            # ---------------- V (token-major) ----------------
            v_all = sb.tile([128, NBLK * 256], MMDT)
            for g in range(8):  # each g covers 4 ctx chunks
                vps = ps.tile([128, 1024], F32, tag="w", name=f"vps{g}")
                for k in range(4):
                    ci = g * 4 + k
                    for i in range(2):
                        nc.tensor.matmul(vps[:, k * 256:(k + 1) * 256],
                                         yslice(i, ci * 128, 128),
                                         wv_t[:, i * 256:(i + 1) * 256],
                                         start=(i == 0), stop=(i == 1))
                dst = v_all[:, g * 1024:(g + 1) * 1024]
                if ev % 2 == 0:
                    nc.scalar.copy(dst, vps)
                else:
                    nc.vector.tensor_copy(dst, vps)
                ev += 1

            # ---------------- attention ----------------
            att2 = ps.tile([128, 2 * QS], F32, tag="att", bufs=1, name="att2")
            csum = ps.tile([2, QS], F32, tag="csum", bufs=1, name="csum")

            def attn_score(j):
                """S^T and exp for context chunk pair j (chunks 2j, 2j+1)"""
                sps = ps.tile([128, 1024], F32, tag="w", name=f"sps{j}")
                for h in range(2):
                    i = 2 * j + h
                    nc.tensor.matmul(sps[:, h * 512:(h + 1) * 512],
                                     kt[0][:, i * 128:(i + 1) * 128], qp2[:, 0:512],
                                     start=True, stop=False)
                    nc.tensor.matmul(sps[:, h * 512:(h + 1) * 512],
                                     kt[1][:, i * 128:(i + 1) * 128], qp2[:, 512:1024],
                                     start=False, stop=True)
                pt = ptp.tile([128, 1024], MMDT, tag="pt", name=f"pt{j}")
                nc.scalar.activation(pt, sps, AF.Exp)
                return pt

            def attn_accum(j, pt):
                """P@V and colsum accumulation for chunk pair j"""
                for h in range(2):
                    i = 2 * j + h
                    first = (i == 0)
                    last = (i == NBLK - 1)
                    for cb in range(2):
                        nc.tensor.matmul(
                            att2[:, cb * 512:(cb + 1) * 512],
                            v_all[:, i * 256 + cb * 128:i * 256 + (cb + 1) * 128],
                            pt[:, h * 512:(h + 1) * 512], start=first, stop=last)
                    nc.tensor.matmul(csum, ones_c, pt[:, h * 512:(h + 1) * 512],
                                     start=first, stop=last)

            # software-pipelined by one pair so the exp latency stays off
            # the PE's static instruction order
            prev = attn_score(0)
            for j in range(1, NPAIR):
                cur = attn_score(j)
                attn_accum(j - 1, prev)
                prev = cur
            attn_accum(NPAIR - 1, prev)

            # softmax normalize: rr = 1/colsum (DVE), broadcast via matmul
            rrf = sb.tile([1, QS], F32)
            nc.vector.reciprocal(rrf, csum[0:1, :])
            rr = sb.tile([1, QS], MMDT)
            nc.vector.tensor_copy(rr, rrf)
            rb2 = ps.tile([128, 1024], F32, tag="w", name="rb2")
            for cb in range(2):
                nc.tensor.matmul(rb2[:, cb * 512:(cb + 1) * 512], ones_r, rr,
                                 start=True, stop=True)
            rbs = sb.tile([128, 1024], MMDT)
            nc.scalar.copy(rbs, rb2)
            attnT2 = sb.tile([128, 1024], MMDT)
            nc.vector.tensor_mul(attnT2, att2, rbs)

            # z = Wo @ attnT + bo', LayerNorm stats
            zps = ps.tile([128, 1024], F32, tag="att", bufs=1, name="zps")
            for cb in range(2):
                nc.tensor.matmul(zps[:, cb * 512:(cb + 1) * 512],
                                 wsl(wo_t, 0, cb), attnT2[:, 0:512],
                                 start=True, stop=False)
                nc.tensor.matmul(zps[:, cb * 512:(cb + 1) * 512],
                                 wsl(wo_t, 1, cb), attnT2[:, 512:1024],
                                 start=False, stop=True)
            zs2 = sb.tile([128, 1024], MMDT)
            for cb in range(2):
                nc.scalar.activation(zs2[:, cb * 512:(cb + 1) * 512],
                                     zps[:, cb * 512:(cb + 1) * 512], AF.Identity,
                                     bias=bvec[cb][:, 1:2])
            zsq2 = sb.tile([128, 1024], MMDT)
            nc.vector.tensor_mul(zsq2, zs2, zs2)

            szp = ps.tile([2, QS], F32, tag="csum", bufs=1, name="szp")
            nc.tensor.matmul(szp, ones_c, zs2[:, 0:512], start=True, stop=False)
            nc.tensor.matmul(szp, ones_c, zs2[:, 512:1024], start=False, stop=True)
            sqp = ps.tile([2, QS], F32, tag="w1", bufs=1, name="sqp")
            nc.tensor.matmul(sqp, ones_c, zsq2[:, 0:512], start=True, stop=False)
            nc.tensor.matmul(sqp, ones_c, zsq2[:, 512:1024], start=False, stop=True)

            s2 = sb.tile([1, QS], F32)
            nc.scalar.square(s2, szp[0:1, :])
            var = sb.tile([1, QS], F32)
            nc.vector.scalar_tensor_tensor(var, sqp[0:1, :], float(C), s2,
                                           op0=ALU.mult, op1=ALU.subtract)
            lnv = sb.tile([1, QS], F32)
            nc.scalar.activation(lnv, var, AF.Ln, bias=eps2v)
            rstd = sb.tile([1, QS], MMDT)
            nc.scalar.activation(rstd, lnv, AF.Exp, scale=-0.5, bias=lnCv)
            # last ln/exp use -- switch the act table to the gelu set now so the
            # ~2.7us load hides behind the LN broadcast/apply + W1 matmuls.
            load_table(GELU_SET)
            neg_mean = sb.tile([1, QS], F32)
            nc.vector.tensor_scalar_mul(neg_mean, szp[0:1, :], -1.0 / C)
            nmrs = sb.tile([1, QS], MMDT)
            nc.vector.tensor_mul(nmrs, neg_mean, rstd)

            rstd_b2 = ps.tile([128, 1024], F32, tag="w", name="rstd_b2")
            nmrs_b2 = ps.tile([128, 1024], F32, tag="w", name="nmrs_b2")
            for cb in range(2):
                nc.tensor.matmul(rstd_b2[:, cb * 512:(cb + 1) * 512], ones_r, rstd,
                                 start=True, stop=True)
                nc.tensor.matmul(nmrs_b2[:, cb * 512:(cb + 1) * 512], ones_r, nmrs,
                                 start=True, stop=True)

            zt2 = sb.tile([128, 1024], MMDT)
            nc.vector.tensor_mul(zt2, zs2, rstd_b2)
            zln2 = sb.tile([128, 1024], MMDT)
            nc.vector.tensor_add(zln2, zt2, nmrs_b2)

            # MLP + residual
            hs = [sb.tile([128, QS], MMDT, name=f"hs{i}") for i in range(4)]
            for g in range(2):  # hidden-block pairs (0,1) and (2,3)
                hps = ps.tile([128, 1024], F32, tag="w", name=f"hps{g}")
                for h in range(2):
                    hb = 2 * g + h
                    nc.tensor.matmul(hps[:, h * 512:(h + 1) * 512],
                                     wsl(w1_t, 0, hb), zln2[:, 0:512],
                                     start=True, stop=False)
                    nc.tensor.matmul(hps[:, h * 512:(h + 1) * 512],
                                     wsl(w1_t, 1, hb), zln2[:, 512:1024],
                                     start=False, stop=True)
                for h in range(2):
                    hb = 2 * g + h
                    nc.scalar.activation(hs[hb], hps[:, h * 512:(h + 1) * 512],
                                         AF.Gelu, bias=b1p[:, hb:hb + 1])

            tps2 = ps.tile([128, 1024], F32, tag="att", bufs=1, name="tps2")
            for cb in range(2):
                for hb in range(4):
                    nc.tensor.matmul(
                        tps2[:, cb * 512:(cb + 1) * 512],
                        w2_t[:, hb * 256 + cb * 128:hb * 256 + (cb + 1) * 128],
                        hs[hb], start=(hb == 0), stop=(hb == 3))
            ot = [sb.tile([128, QS], F32, name=f"ot{i}") for i in range(2)]
            for cb in range(2):
                nc.vector.scalar_tensor_tensor(
                    ot[cb], tps2[:, cb * 512:(cb + 1) * 512], bvec[cb][:, 2:3],
                    xf[cb], op0=ALU.add, op1=ALU.add)
            nc.sync.dma_start(d_out[0:128, :], ot[0])
            nc.gpsimd.dma_start(d_out[128:256, :], ot[1])

    nc.compile()
    return nc


_NC = None


def _get_nc():
    global _NC
    if _NC is None:
        _NC = _build_nc()
    return _NC


def _pack_rows(a, nchunk):
    """(nchunk*128, W) -> (128, nchunk*W) with row-chunks side by side."""
    w = a.shape[1]
    out = np.empty((128, nchunk * w), a.dtype)
    for i in range(nchunk):
        out[:, i * w:(i + 1) * w] = a[i * 128:(i + 1) * 128, :]
    return out


def prep_in_maps(x, y, Wq, bq, Wk, bk, Wv, bv, Wo, bo, ln_w, ln_b, W1, b1, W2, b2):
    f = lambda a: np.asarray(a, dtype=np.float32)
    x, y = f(x), f(y)
    Wq, bq, Wk, Wv, bv, Wo, bo = f(Wq), f(bq), f(Wk), f(Wv), f(bv), f(Wo), f(bo)
    ln_w, ln_b, W1, b1, W2, b2 = f(ln_w), f(ln_b), f(W1), f(b1), f(W2), f(b2)

    mmnp = mybir.dt.np(MMDT)
    g = lambda a: np.ascontiguousarray(a).astype(mmnp)

    x_cm = np.ascontiguousarray(x.reshape(C, HW))
    y_cm = np.ascontiguousarray(y.reshape(C, NCTX))

    # host-side algebraic folds
    bo_p = (Wo.astype(np.float64) @ bv.astype(np.float64) + bo).astype(np.float32)
    b1_p = (W1.astype(np.float64) @ ln_b.astype(np.float64) + b1).astype(np.float32)
    W1p = (W1 * ln_w[None, :]).astype(np.float32)

    bvec = np.stack([bq / 16.0, bo_p, b2], axis=1).astype(np.float32)  # (256,3)

    common = {
        "y_mm": g(_pack_rows(y_cm, 2)),
        "wq_mm": g(_pack_rows(Wq.T, 2)),
        "wk_mm": g(_pack_rows(Wk.T, 2)),
        "wv_mm": g(_pack_rows(Wv.T, 2)),
        "wo_mm": g(_pack_rows(Wo.T, 2)),
        "w1_mm": g(_pack_rows(W1p.T, 2)),
        "w2_mm": g(_pack_rows(W2.T, 4)),
        "bvec": bvec,
        "b1p": np.ascontiguousarray(b1_p.reshape(4, 128).T),
        "ones_c": np.ones((128, 2), mmnp),
        "ones_r": np.ones((1, 128), mmnp),
    }
    in_maps = []
    for i in range(NCORES):
        m = dict(common)
        xs = np.ascontiguousarray(x_cm[:, i * QS:(i + 1) * QS])
        m["x_f32"] = xs
        m["x_mm"] = g(_pack_rows(xs, 2))
        in_maps.append(m)
    return in_maps


def kernel(**inputs):
    in_maps = prep_in_maps(**inputs)
    nc = _get_nc()
    res = bass_utils.run_bass_kernel_spmd(nc, in_maps, core_ids=list(range(NCORES)))
    t = np.concatenate([res.results[i]["out_sh"] for i in range(NCORES)], axis=1)
    return t.reshape(1, C, 64, 64)


# revision 4
# speedup vs baseline: 1.1514x; 1.1215x over previous
"""Trainium2 Bass kernel for nn_CMEncoder (cross-attention + LayerNorm2d + MLP block).

Strategy (8 NeuronCores, sequence-parallel over the HW=4096 query tokens):
  - Each core owns 512 query tokens; K/V over the full 4096-token context are
    computed redundantly on every core (no collectives needed).
  - Everything stays channel-major on chip ([feature partition, token free]).
  - Scores are computed transposed (S^T[n, q]) so P = exp(S^T) is the moving
    operand of the P@V matmuls (att^T = V^T @ P); the softmax denominator
    comes from a cheap ones-stationary reduction matmul.
  - All matmuls run in bf16 (2.4 GHz streaming + FWL weight loads, vs 1.2 GHz
    and no FWL for fp32) with fp32 PSUM accumulation.
  - The attention loop is software-pipelined 3 deep so the softmax-exp (ACT)
    latency never blocks the PE's static instruction order.
  - PSUM->SBUF evacuations are split between the scalar and vector engines.
  - Host-side algebraic folds: bk dropped (softmax shift invariance), bv folded
    into the output-projection bias, the 1/sqrt(C) scale folded into the Q
    bias/scale, LayerNorm's affine folded into the MLP's first layer.
  - Only two act-table loads: ln/exp at start, gelu right before the MLP
    (auto-inserted, hidden behind the W1 matmuls).
"""

import math
import numpy as np
import concourse.bacc as bacc
import concourse.mybir as mybir
import concourse.tile as tile
from concourse import bass_utils
from concourse.hw_specs import get_activation_tables

F32 = mybir.dt.float32
BF16 = mybir.dt.bfloat16
AF = mybir.ActivationFunctionType
ALU = mybir.AluOpType

MMDT = BF16      # matmul operand dtype

C = 256          # channels
HW = 4096        # query tokens (64x64)
NCTX = 4096      # context tokens
HID = 512        # mlp hidden
NCORES = 8
QS = HW // NCORES   # 512 queries per core
NBLK = NCTX // 128  # 32 context chunks
EPS = 1e-6


def _build_nc():
    nc = bacc.Bacc("TRN2", target_bir_lowering=False)

    # --- DRAM I/O (weights pre-packed on host: row-chunks side by side) ---
    d_xmm = nc.dram_tensor("x_mm", (128, 2 * QS), MMDT, kind="ExternalInput")
    d_xf = nc.dram_tensor("x_f32", (C, QS), F32, kind="ExternalInput")
    d_y = nc.dram_tensor("y_mm", (128, 2 * NCTX), MMDT, kind="ExternalInput")
    d_wq = nc.dram_tensor("wq_mm", (128, 2 * C), MMDT, kind="ExternalInput")
    d_wk = nc.dram_tensor("wk_mm", (128, 2 * C), MMDT, kind="ExternalInput")
    d_wv = nc.dram_tensor("wv_mm", (128, 2 * C), MMDT, kind="ExternalInput")
    d_wo = nc.dram_tensor("wo_mm", (128, 2 * C), MMDT, kind="ExternalInput")
    d_w1 = nc.dram_tensor("w1_mm", (128, 2 * HID), MMDT, kind="ExternalInput")
    d_w2 = nc.dram_tensor("w2_mm", (128, 4 * C), MMDT, kind="ExternalInput")
    d_bv = nc.dram_tensor("bvec", (C, 3), F32, kind="ExternalInput")   # [bq/16, bo', b2]
    d_b1 = nc.dram_tensor("b1p", (128, 4), F32, kind="ExternalInput")
    d_oc = nc.dram_tensor("ones_c", (128, 2), MMDT, kind="ExternalInput")
    d_or = nc.dram_tensor("ones_r", (1, 128), MMDT, kind="ExternalInput")
    d_out = nc.dram_tensor("out_sh", (C, QS), F32, kind="ExternalOutput")

    tabs = list(get_activation_tables(nc.m.arch).keys())
    LNEXP_SET = tabs.index("natural_log_exp_and_others")

    with tile.TileContext(nc) as tc:
        # Pre-load the exp+ln activation table once; the gelu set is
        # auto-loaded right before the MLP's gelu (the only other set used).
        nc.scalar.add_instruction(mybir.InstLoadActFuncSet(
            name=nc.get_next_instruction_name(), ins=[], outs=[],
            act_func_set_id=LNEXP_SET))

        with (
            tc.tile_pool(name="sb", bufs=1) as sb,
            tc.tile_pool(name="pt_pool", bufs=4) as ptp,
            tc.tile_pool(name="ps", bufs=4, space="PSUM") as ps,
        ):
            # ---------------- input DMAs ----------------
            # sync queue: the tensors the PE needs first, in need-order.
            wq_t = sb.tile([128, 2 * C], MMDT)
            nc.sync.dma_start(wq_t, d_wq[:, :])
            xmm = sb.tile([128, 2 * QS], MMDT)
            nc.sync.dma_start(xmm[:, 0:QS], d_xmm[:, 0:QS])
            nc.sync.dma_start(xmm[:, QS:2 * QS], d_xmm[:, QS:2 * QS])
            wk_t = sb.tile([128, 2 * C], MMDT)
            nc.sync.dma_start(wk_t, d_wk[:, :])
            # y in ctx-quarters, both channel-halves of a quarter back to back
            yq = [[None] * 4 for _ in range(2)]
            for q in range(4):
                for i in range(2):
                    yq[i][q] = sb.tile([128, 1024], MMDT, name=f"y{i}{q}")
                    nc.sync.dma_start(
                        yq[i][q], d_y[:, i * NCTX + q * 1024:i * NCTX + (q + 1) * 1024])

            # gpsimd queue: everything needed later.
            wv_t = sb.tile([128, 2 * C], MMDT)
            nc.gpsimd.dma_start(wv_t, d_wv[:, :])
            ones_c = sb.tile([128, 2], MMDT)
            nc.gpsimd.dma_start(ones_c, d_oc[:, :])
            ones_r = sb.tile([1, 128], MMDT)
            nc.gpsimd.dma_start(ones_r, d_or[:, :])
            bvec = [sb.tile([128, 3], F32, name=f"bvec{i}") for i in range(2)]
            for i in range(2):
                nc.gpsimd.dma_start(bvec[i], d_bv[i * 128:(i + 1) * 128, :])
            wo_t = sb.tile([128, 2 * C], MMDT)
            nc.gpsimd.dma_start(wo_t, d_wo[:, :])
            w1_t = sb.tile([128, 2 * HID], MMDT)
            nc.gpsimd.dma_start(w1_t, d_w1[:, :])
            w2_t = sb.tile([128, 4 * C], MMDT)
            nc.gpsimd.dma_start(w2_t, d_w2[:, :])
            b1p = sb.tile([128, 4], F32)
            nc.gpsimd.dma_start(b1p, d_b1[:, :])
            xf = [sb.tile([128, QS], F32, name=f"xf{i}") for i in range(2)]
            for i in range(2):
                nc.gpsimd.dma_start(xf[i], d_xf[i * 128:(i + 1) * 128, :])

            eps2v = sb.tile([1, 1], F32)
            nc.vector.memset(eps2v, float(C) * float(C) * EPS)
            lnCv = sb.tile([1, 1], F32)
            nc.vector.memset(lnCv, math.log(float(C)))

            def wsl(t, cc, cb, w=128):
                # packed weight tile slice: row-chunk cc, col-chunk cb
                return t[:, cc * (t.shape[1] // 2) + cb * w:
                         cc * (t.shape[1] // 2) + (cb + 1) * w]

            def yslice(i, c0, w):
                # y channel-half i, ctx cols [c0, c0+w) (must stay in a quarter)
                q, o = c0 // 1024, c0 % 1024
                return yq[i][q][:, o:o + w]

            # ---------------- Q' = (x^T Wq^T + bq)/16, channel-major ----------------
            qp2 = sb.tile([128, 2 * QS], MMDT)
            for cb in range(2):
                qps = ps.tile([128, 512], F32, tag="w", name=f"qps{cb}")
                nc.tensor.matmul(qps, wsl(wq_t, 0, cb), xmm[:, 0:QS],
                                 start=True, stop=False)
                nc.tensor.matmul(qps, wsl(wq_t, 1, cb), xmm[:, QS:2 * QS],
                                 start=False, stop=True)
                nc.scalar.activation(qp2[:, cb * 512:(cb + 1) * 512], qps,
                                     AF.Identity, bias=bvec[cb][:, 0:1],
                                     scale=1.0 / 16.0)

            # ---------------- K^T (channel-major) ----------------
            kt = [sb.tile([128, NCTX], MMDT, name=f"kt{i}") for i in range(2)]
            ev = 0  # evacuation engine round-robin
            for hh in range(4):
                for cb in range(2):
                    c0 = hh * 1024
                    for h in range(2):
                        kps = ps.tile([128, 512], F32, tag="w", name=f"kps{hh}{cb}{h}")
                        nc.tensor.matmul(kps, wsl(wk_t, 0, cb),
                                         yslice(0, c0 + h * 512, 512),
                                         start=True, stop=False)
                        nc.tensor.matmul(kps, wsl(wk_t, 1, cb),
                                         yslice(1, c0 + h * 512, 512),
                                         start=False, stop=True)
                        dst = kt[cb][:, c0 + h * 512:c0 + (h + 1) * 512]
                        if ev % 2 == 0:
                            nc.scalar.copy(dst, kps)
                        else:
                            nc.vector.tensor_copy(dst, kps)
                        ev += 1

            # ---------------- V (token-major) ----------------
            v_all = sb.tile([128, NBLK * 256], MMDT)
            for g in range(16):  # each g covers 2 ctx chunks
                vps = ps.tile([128, 512], F32, tag="w", name=f"vps{g}")
                for k in range(2):
                    ci = g * 2 + k
                    for i in range(2):
                        nc.tensor.matmul(vps[:, k * 256:(k + 1) * 256],
                                         yslice(i, ci * 128, 128),
                                         wv_t[:, i * 256:(i + 1) * 256],
                                         start=(i == 0), stop=(i == 1))
                dst = v_all[:, g * 512:(g + 1) * 512]
                if ev % 2 == 0:
                    nc.scalar.copy(dst, vps)
                else:
                    nc.vector.tensor_copy(dst, vps)
                ev += 1

            # ---------------- attention ----------------
            att2 = ps.tile([128, 2 * QS], F32, tag="att", bufs=1, name="att2")
            csum = ps.tile([2, QS], F32, tag="csum", bufs=1, name="csum")

            def attn_score(i):
                """S^T and exp for context chunk i"""
                sps = ps.tile([128, QS], F32, tag="w", name=f"sps{i}")
                nc.tensor.matmul(sps, kt[0][:, i * 128:(i + 1) * 128],
                                 qp2[:, 0:512], start=True, stop=False)
                nc.tensor.matmul(sps, kt[1][:, i * 128:(i + 1) * 128],
                                 qp2[:, 512:1024], start=False, stop=True)
                pt = ptp.tile([128, QS], MMDT, tag="pt", name=f"pt{i}")
                nc.scalar.activation(pt, sps, AF.Exp)
                return pt

            def attn_accum(i, pt):
                """P@V and colsum accumulation for chunk i"""
                first, last = (i == 0), (i == NBLK - 1)
                for cb in range(2):
                    nc.tensor.matmul(
                        att2[:, cb * 512:(cb + 1) * 512],
                        v_all[:, i * 256 + cb * 128:i * 256 + (cb + 1) * 128],
                        pt, start=first, stop=last)
                nc.tensor.matmul(csum, ones_c, pt, start=first, stop=last)

            # ---- software-pipelined 3 deep: the exp of chunk i completes
            # ---- while the PE runs chunks i+1 / i+2
            p0 = attn_score(0)
            p1 = attn_score(1)
            for i in range(2, NBLK):
                p2 = attn_score(i)
                attn_accum(i - 2, p0)
                p0, p1 = p1, p2
            attn_accum(NBLK - 2, p0)
            attn_accum(NBLK - 1, p1)

            # softmax normalize: 1/colsum via exp(-ln(x)) on ACT
            lncs = sb.tile([1, QS], F32)
            nc.scalar.activation(lncs, csum[0:1, :], AF.Ln)
            rr = sb.tile([1, QS], MMDT)
            nc.scalar.activation(rr, lncs, AF.Exp, scale=-1.0)
            rb = ps.tile([128, QS], F32, tag="w", name="rb")
            nc.tensor.matmul(rb, ones_r, rr, start=True, stop=True)
            rbs = sb.tile([128, QS], MMDT)
            nc.scalar.copy(rbs, rb)
            attnT2 = sb.tile([128, 1024], MMDT)
            for cb in range(2):
                nc.vector.tensor_mul(attnT2[:, cb * 512:(cb + 1) * 512],
                                     att2[:, cb * 512:(cb + 1) * 512], rbs)

            # z = Wo @ attnT + bo', LayerNorm stats
            zps = ps.tile([128, 1024], F32, tag="att", bufs=1, name="zps")
            for cb in range(2):
                nc.tensor.matmul(zps[:, cb * 512:(cb + 1) * 512],
                                 wsl(wo_t, 0, cb), attnT2[:, 0:512],
                                 start=True, stop=False)
                nc.tensor.matmul(zps[:, cb * 512:(cb + 1) * 512],
                                 wsl(wo_t, 1, cb), attnT2[:, 512:1024],
                                 start=False, stop=True)
            zs2 = sb.tile([128, 1024], MMDT)
            for cb in range(2):
                nc.scalar.activation(zs2[:, cb * 512:(cb + 1) * 512],
                                     zps[:, cb * 512:(cb + 1) * 512], AF.Identity,
                                     bias=bvec[cb][:, 1:2])
            zsq2 = sb.tile([128, 1024], MMDT)
            nc.vector.tensor_mul(zsq2, zs2, zs2)

            szp = ps.tile([2, QS], F32, tag="csum", bufs=1, name="szp")
            nc.tensor.matmul(szp, ones_c, zs2[:, 0:512], start=True, stop=False)
            nc.tensor.matmul(szp, ones_c, zs2[:, 512:1024], start=False, stop=True)
            sqp = ps.tile([2, QS], F32, tag="w", name="sqp")
            nc.tensor.matmul(sqp, ones_c, zsq2[:, 0:512], start=True, stop=False)
            nc.tensor.matmul(sqp, ones_c, zsq2[:, 512:1024], start=False, stop=True)

            s2 = sb.tile([1, QS], F32)
            nc.scalar.square(s2, szp[0:1, :])
            var = sb.tile([1, QS], F32)
            nc.vector.scalar_tensor_tensor(var, sqp[0:1, :], float(C), s2,
                                           op0=ALU.mult, op1=ALU.subtract)
            lnv = sb.tile([1, QS], F32)
            nc.scalar.activation(lnv, var, AF.Ln, bias=eps2v)
            rstd = sb.tile([1, QS], MMDT)
            nc.scalar.activation(rstd, lnv, AF.Exp, scale=-0.5, bias=lnCv)
            neg_mean = sb.tile([1, QS], F32)
            nc.vector.tensor_scalar_mul(neg_mean, szp[0:1, :], -1.0 / C)
            nmrs = sb.tile([1, QS], MMDT)
            nc.vector.tensor_mul(nmrs, neg_mean, rstd)

            rstd_b = ps.tile([128, QS], F32, tag="w", name="rstd_b")
            nc.tensor.matmul(rstd_b, ones_r, rstd, start=True, stop=True)
            nmrs_b = ps.tile([128, QS], F32, tag="w", name="nmrs_b")
            nc.tensor.matmul(nmrs_b, ones_r, nmrs, start=True, stop=True)

            zln2 = sb.tile([128, 1024], MMDT)
            for cb in range(2):
                zt = sb.tile([128, QS], MMDT, name=f"zt{cb}")
                nc.vector.tensor_mul(zt, zs2[:, cb * 512:(cb + 1) * 512], rstd_b)
                nc.vector.tensor_add(zln2[:, cb * 512:(cb + 1) * 512], zt, nmrs_b)

            # MLP + residual
            hs = [sb.tile([128, QS], MMDT, name=f"hs{i}") for i in range(4)]
            for hb in range(4):
                hps = ps.tile([128, QS], F32, tag="w", name=f"hps{hb}")
                nc.tensor.matmul(hps, wsl(w1_t, 0, hb), zln2[:, 0:512],
                                 start=True, stop=False)
                nc.tensor.matmul(hps, wsl(w1_t, 1, hb), zln2[:, 512:1024],
                                 start=False, stop=True)
                nc.scalar.activation(hs[hb], hps, AF.Gelu, bias=b1p[:, hb:hb + 1])

            tps2 = ps.tile([128, 1024], F32, tag="att", bufs=1, name="tps2")
            for cb in range(2):
                for hb in range(4):
                    nc.tensor.matmul(
                        tps2[:, cb * 512:(cb + 1) * 512],
                        w2_t[:, hb * 256 + cb * 128:hb * 256 + (cb + 1) * 128],
                        hs[hb], start=(hb == 0), stop=(hb == 3))
            ot = [sb.tile([128, QS], F32, name=f"ot{i}") for i in range(2)]
            for cb in range(2):
                nc.vector.scalar_tensor_tensor(
                    ot[cb], tps2[:, cb * 512:(cb + 1) * 512], bvec[cb][:, 2:3],
                    xf[cb], op0=ALU.add, op1=ALU.add)
            # spread the output store over four DMA queues
            nc.sync.dma_start(d_out[0:128, 0:256], ot[0][:, 0:256])
            nc.scalar.dma_start(d_out[0:128, 256:512], ot[0][:, 256:512])
            nc.gpsimd.dma_start(d_out[128:256, 0:256], ot[1][:, 0:256])
            nc.sync.dma_start(d_out[128:256, 256:512], ot[1][:, 256:512])

    nc.compile()
    return nc


_NC = None


def _get_nc():
    global _NC
    if _NC is None:
        _NC = _build_nc()
    return _NC


def _pack_rows(a, nchunk):
    """(nchunk*128, W) -> (128, nchunk*W) with row-chunks side by side."""
    w = a.shape[1]
    out = np.empty((128, nchunk * w), a.dtype)
    for i in range(nchunk):
        out[:, i * w:(i + 1) * w] = a[i * 128:(i + 1) * 128, :]
    return out


def prep_in_maps(x, y, Wq, bq, Wk, bk, Wv, bv, Wo, bo, ln_w, ln_b, W1, b1, W2, b2):
    f = lambda a: np.asarray(a, dtype=np.float32)
    x, y = f(x), f(y)
    Wq, bq, Wk, Wv, bv, Wo, bo = f(Wq), f(bq), f(Wk), f(Wv), f(bv), f(Wo), f(bo)
    ln_w, ln_b, W1, b1, W2, b2 = f(ln_w), f(ln_b), f(W1), f(b1), f(W2), f(b2)

    mmnp = mybir.dt.np(MMDT)
    g = lambda a: np.ascontiguousarray(a).astype(mmnp)

    x_cm = np.ascontiguousarray(x.reshape(C, HW))
    y_cm = np.ascontiguousarray(y.reshape(C, NCTX))

    # host-side algebraic folds
    bo_p = (Wo.astype(np.float64) @ bv.astype(np.float64) + bo).astype(np.float32)
    b1_p = (W1.astype(np.float64) @ ln_b.astype(np.float64) + b1).astype(np.float32)
    W1p = (W1 * ln_w[None, :]).astype(np.float32)

    bvec = np.stack([bq / 16.0, bo_p, b2], axis=1).astype(np.float32)  # (256,3)

    common = {
        "y_mm": g(_pack_rows(y_cm, 2)),
        "wq_mm": g(_pack_rows(Wq.T, 2)),
        "wk_mm": g(_pack_rows(Wk.T, 2)),
        "wv_mm": g(_pack_rows(Wv.T, 2)),
        "wo_mm": g(_pack_rows(Wo.T, 2)),
        "w1_mm": g(_pack_rows(W1p.T, 2)),
        "w2_mm": g(_pack_rows(W2.T, 4)),
        "bvec": bvec,
        "b1p": np.ascontiguousarray(b1_p.reshape(4, 128).T),
        "ones_c": np.ones((128, 2), mmnp),
        "ones_r": np.ones((1, 128), mmnp),
    }
    in_maps = []
    for i in range(NCORES):
        m = dict(common)
        xs = np.ascontiguousarray(x_cm[:, i * QS:(i + 1) * QS])
        m["x_f32"] = xs
        m["x_mm"] = g(_pack_rows(xs, 2))
        in_maps.append(m)
    return in_maps


def kernel(**inputs):
    in_maps = prep_in_maps(**inputs)
    nc = _get_nc()
    res = bass_utils.run_bass_kernel_spmd(nc, in_maps, core_ids=list(range(NCORES)))
    t = np.concatenate([res.results[i]["out_sh"] for i in range(NCORES)], axis=1)
    return t.reshape(1, C, 64, 64)


# revision 6
# speedup vs baseline: 1.2324x; 1.0703x over previous
"""Trainium2 Bass kernel for nn_CMEncoder (cross-attention + LayerNorm2d + MLP block).

Strategy (8 NeuronCores, sequence-parallel over the HW=4096 query tokens):
  - Each core owns 512 query tokens; K/V over the full 4096-token context are
    computed redundantly on every core (no collectives needed).
  - Everything stays channel-major on chip ([feature partition, token free]).
  - Scores are computed transposed (S^T[n, q]) so P = exp(S^T) is the moving
    operand of the P@V matmuls (att^T = V^T @ P); the softmax denominator
    comes from a cheap ones-stationary reduction matmul.
  - All matmuls run in bf16 (2.4 GHz streaming + FWL weight loads, vs 1.2 GHz
    and no FWL for fp32) with fp32 PSUM accumulation.
  - The attention loop is software-pipelined 3 deep so the softmax-exp (ACT)
    latency never blocks the PE's static instruction order.
  - PSUM->SBUF evacuations are split between the scalar and vector engines.
  - Host-side algebraic folds: bk dropped (softmax shift invariance), bv folded
    into the output-projection bias, the 1/sqrt(C) scale folded into the Q
    bias/scale, LayerNorm's affine folded into the MLP's first layer.
  - Only two act-table loads: ln/exp at start, gelu right before the MLP
    (auto-inserted, hidden behind the W1 matmuls).
"""

import math
import numpy as np
import concourse.bacc as bacc
import concourse.mybir as mybir
import concourse.tile as tile
from concourse import bass_utils
from concourse.hw_specs import get_activation_tables

F32 = mybir.dt.float32
BF16 = mybir.dt.bfloat16
AF = mybir.ActivationFunctionType
ALU = mybir.AluOpType

MMDT = BF16      # matmul operand dtype

C = 256          # channels
HW = 4096        # query tokens (64x64)
NCTX = 4096      # context tokens
HID = 512        # mlp hidden
NCORES = 8
QS = HW // NCORES   # 512 queries per core
NBLK = NCTX // 128  # 32 context chunks
EPS = 1e-6


def _build_nc():
    nc = bacc.Bacc("TRN2", target_bir_lowering=False)

    # --- DRAM I/O (weights pre-packed on host: row-chunks side by side) ---
    d_xmm = nc.dram_tensor("x_mm", (128, 2 * QS), MMDT, kind="ExternalInput")
    d_xf = nc.dram_tensor("x_f32", (C, QS), F32, kind="ExternalInput")
    d_y = nc.dram_tensor("y_mm", (128, 2 * NCTX), MMDT, kind="ExternalInput")
    d_wq = nc.dram_tensor("wq_mm", (128, 2 * C), MMDT, kind="ExternalInput")
    d_wk = nc.dram_tensor("wk_mm", (128, 2 * C), MMDT, kind="ExternalInput")
    d_wv = nc.dram_tensor("wv_mm", (128, 2 * C), MMDT, kind="ExternalInput")
    d_wo = nc.dram_tensor("wo_mm", (128, 2 * C), MMDT, kind="ExternalInput")
    d_w1 = nc.dram_tensor("w1_mm", (128, 2 * HID), MMDT, kind="ExternalInput")
    d_w2 = nc.dram_tensor("w2_mm", (128, 4 * C), MMDT, kind="ExternalInput")
    d_bv = nc.dram_tensor("bvec", (C, 3), F32, kind="ExternalInput")   # [bq/16, bo', b2]
    d_b1 = nc.dram_tensor("b1p", (128, 4), F32, kind="ExternalInput")
    d_oc = nc.dram_tensor("ones_c", (128, 128), MMDT, kind="ExternalInput")
    d_or = nc.dram_tensor("ones_r", (1, 128), MMDT, kind="ExternalInput")
    d_out = nc.dram_tensor("out_sh", (C, QS), F32, kind="ExternalOutput")

    tabs = list(get_activation_tables(nc.m.arch).keys())
    LNEXP_SET = tabs.index("natural_log_exp_and_others")

    with tile.TileContext(nc) as tc:
        # Pre-load the exp+ln activation table once; the gelu set is
        # auto-loaded right before the MLP's gelu (the only other set used).
        nc.scalar.add_instruction(mybir.InstLoadActFuncSet(
            name=nc.get_next_instruction_name(), ins=[], outs=[],
            act_func_set_id=LNEXP_SET))

        with (
            tc.tile_pool(name="sb", bufs=1) as sb,
            tc.tile_pool(name="pt_pool", bufs=4) as ptp,
            tc.tile_pool(name="ps", bufs=4, space="PSUM") as ps,
        ):
            # ---------------- input DMAs ----------------
            # sync queue: the tensors the PE needs first, in need-order.
            wq_t = sb.tile([128, 2 * C], MMDT)
            nc.sync.dma_start(wq_t, d_wq[:, :])
            xmm = sb.tile([128, 2 * QS], MMDT)
            nc.sync.dma_start(xmm[:, 0:QS], d_xmm[:, 0:QS])
            nc.sync.dma_start(xmm[:, QS:2 * QS], d_xmm[:, QS:2 * QS])
            wk_t = sb.tile([128, 2 * C], MMDT)
            nc.sync.dma_start(wk_t, d_wk[:, :])
            # y in ctx-quarters, both channel-halves of a quarter back to back
            yq = [[None] * 4 for _ in range(2)]
            for q in range(4):
                for i in range(2):
                    yq[i][q] = sb.tile([128, 1024], MMDT, name=f"y{i}{q}")
                    nc.sync.dma_start(
                        yq[i][q], d_y[:, i * NCTX + q * 1024:i * NCTX + (q + 1) * 1024])

            # gpsimd queue: everything needed later.
            wv_t = sb.tile([128, 2 * C], MMDT)
            nc.gpsimd.dma_start(wv_t, d_wv[:, :])
            ones_c = sb.tile([128, 128], MMDT)
            nc.gpsimd.dma_start(ones_c, d_oc[:, :])
            ones_r = sb.tile([1, 128], MMDT)
            nc.gpsimd.dma_start(ones_r, d_or[:, :])
            bvec = [sb.tile([128, 3], F32, name=f"bvec{i}") for i in range(2)]
            for i in range(2):
                nc.gpsimd.dma_start(bvec[i], d_bv[i * 128:(i + 1) * 128, :])
            wo_t = sb.tile([128, 2 * C], MMDT)
            nc.gpsimd.dma_start(wo_t, d_wo[:, :])
            w1_t = sb.tile([128, 2 * HID], MMDT)
            nc.gpsimd.dma_start(w1_t, d_w1[:, :])
            w2_t = sb.tile([128, 4 * C], MMDT)
            nc.gpsimd.dma_start(w2_t, d_w2[:, :])
            b1p = sb.tile([128, 4], F32)
            nc.gpsimd.dma_start(b1p, d_b1[:, :])
            xf = [sb.tile([128, QS], F32, name=f"xf{i}") for i in range(2)]
            for i in range(2):
                nc.gpsimd.dma_start(xf[i], d_xf[i * 128:(i + 1) * 128, :])

            eps2v = sb.tile([1, 1], F32)
            nc.vector.memset(eps2v, float(C) * float(C) * EPS)
            lnCv = sb.tile([1, 1], F32)
            nc.vector.memset(lnCv, math.log(float(C)))

            def wsl(t, cc, cb, w=128):
                # packed weight tile slice: row-chunk cc, col-chunk cb
                return t[:, cc * (t.shape[1] // 2) + cb * w:
                         cc * (t.shape[1] // 2) + (cb + 1) * w]

            def yslice(i, c0, w):
                # y channel-half i, ctx cols [c0, c0+w) (must stay in a quarter)
                q, o = c0 // 1024, c0 % 1024
                return yq[i][q][:, o:o + w]

            # ---------------- Q' = (x^T Wq^T + bq)/16, channel-major ----------------
            qp2 = sb.tile([128, 2 * QS], MMDT)
            for cb in range(2):
                qps = ps.tile([128, 512], F32, tag="w", name=f"qps{cb}")
                nc.tensor.matmul(qps, wsl(wq_t, 0, cb), xmm[:, 0:QS],
                                 start=True, stop=False)
                nc.tensor.matmul(qps, wsl(wq_t, 1, cb), xmm[:, QS:2 * QS],
                                 start=False, stop=True)
                nc.scalar.activation(qp2[:, cb * 512:(cb + 1) * 512], qps,
                                     AF.Identity, bias=bvec[cb][:, 0:1],
                                     scale=1.0 / 16.0)

            # ---------------- K^T (channel-major) ----------------
            kt = [sb.tile([128, NCTX], MMDT, name=f"kt{i}") for i in range(2)]
            ev = 0  # evacuation engine round-robin
            for hh in range(4):
                for cb in range(2):
                    c0 = hh * 1024
                    for h in range(2):
                        kps = ps.tile([128, 512], F32, tag="w", name=f"kps{hh}{cb}{h}")
                        nc.tensor.matmul(kps, wsl(wk_t, 0, cb),
                                         yslice(0, c0 + h * 512, 512),
                                         start=True, stop=False)
                        nc.tensor.matmul(kps, wsl(wk_t, 1, cb),
                                         yslice(1, c0 + h * 512, 512),
                                         start=False, stop=True)
                        dst = kt[cb][:, c0 + h * 512:c0 + (h + 1) * 512]
                        if ev % 2 == 0:
                            nc.scalar.copy(dst, kps)
                        else:
                            nc.vector.tensor_copy(dst, kps)
                        ev += 1

            # ---------------- V (token-major) ----------------
            v_all = sb.tile([128, NBLK * 256], MMDT)
            for g in range(16):  # each g covers 2 ctx chunks
                vps = ps.tile([128, 512], F32, tag="w", name=f"vps{g}")
                for k in range(2):
                    ci = g * 2 + k
                    for i in range(2):
                        nc.tensor.matmul(vps[:, k * 256:(k + 1) * 256],
                                         yslice(i, ci * 128, 128),
                                         wv_t[:, i * 256:(i + 1) * 256],
                                         start=(i == 0), stop=(i == 1))
                dst = v_all[:, g * 512:(g + 1) * 512]
                if ev % 2 == 0:
                    nc.scalar.copy(dst, vps)
                else:
                    nc.vector.tensor_copy(dst, vps)
                ev += 1

            # ---------------- attention ----------------
            att2 = ps.tile([128, 2 * QS], F32, tag="att", bufs=1, name="att2")
            csum = ps.tile([128, QS], F32, tag="csum", bufs=1, name="csum")

            def attn_score(i):
                """S^T and exp for context chunk i"""
                sps = ps.tile([128, QS], F32, tag="w", name=f"sps{i}")
                nc.tensor.matmul(sps, kt[0][:, i * 128:(i + 1) * 128],
                                 qp2[:, 0:512], start=True, stop=False)
                nc.tensor.matmul(sps, kt[1][:, i * 128:(i + 1) * 128],
                                 qp2[:, 512:1024], start=False, stop=True)
                pt = ptp.tile([128, QS], MMDT, tag="pt", name=f"pt{i}")
                nc.scalar.activation(pt, sps, AF.Exp)
                return pt

            def attn_accum(i, pt):
                """P@V and colsum accumulation for chunk i"""
                first, last = (i == 0), (i == NBLK - 1)
                for cb in range(2):
                    nc.tensor.matmul(
                        att2[:, cb * 512:(cb + 1) * 512],
                        v_all[:, i * 256 + cb * 128:i * 256 + (cb + 1) * 128],
                        pt, start=first, stop=last)
                nc.tensor.matmul(csum, ones_c, pt, start=first, stop=last)

            # ---- software-pipelined 3 deep: the exp of chunk i completes
            # ---- while the PE runs chunks i+1 / i+2
            p0 = attn_score(0)
            p1 = attn_score(1)
            for i in range(2, NBLK):
                p2 = attn_score(i)
                attn_accum(i - 2, p0)
                p0, p1 = p1, p2
            attn_accum(NBLK - 2, p0)
            attn_accum(NBLK - 1, p1)

            # softmax normalize: 1/colsum via exp(-ln(x)) on ACT
            lncs = sb.tile([1, QS], F32)
            nc.scalar.activation(lncs, csum[0:1, :], AF.Ln)
            rr = sb.tile([1, QS], MMDT)
            nc.scalar.activation(rr, lncs, AF.Exp, scale=-1.0)
            rb = ps.tile([128, QS], F32, tag="w", name="rb")
            nc.tensor.matmul(rb, ones_r, rr, start=True, stop=True)
            rbs = sb.tile([128, QS], MMDT)
            nc.scalar.copy(rbs, rb)
            attnT2 = sb.tile([128, 1024], MMDT)
            for cb in range(2):
                nc.vector.tensor_mul(attnT2[:, cb * 512:(cb + 1) * 512],
                                     att2[:, cb * 512:(cb + 1) * 512], rbs)

            # z = Wo @ attnT + bo', LayerNorm stats
            zps = ps.tile([128, 1024], F32, tag="att", bufs=1, name="zps")
            for cb in range(2):
                nc.tensor.matmul(zps[:, cb * 512:(cb + 1) * 512],
                                 wsl(wo_t, 0, cb), attnT2[:, 0:512],
                                 start=True, stop=False)
                nc.tensor.matmul(zps[:, cb * 512:(cb + 1) * 512],
                                 wsl(wo_t, 1, cb), attnT2[:, 512:1024],
                                 start=False, stop=True)
            zs2 = sb.tile([128, 1024], MMDT)
            for cb in range(2):
                nc.scalar.activation(zs2[:, cb * 512:(cb + 1) * 512],
                                     zps[:, cb * 512:(cb + 1) * 512], AF.Identity,
                                     bias=bvec[cb][:, 1:2])
            zsq2 = sb.tile([128, 1024], MMDT)
            nc.vector.tensor_mul(zsq2, zs2, zs2)

            szp = ps.tile([128, QS], F32, tag="csum", bufs=1, name="szp")
            nc.tensor.matmul(szp, ones_c, zs2[:, 0:512], start=True, stop=False)
            nc.tensor.matmul(szp, ones_c, zs2[:, 512:1024], start=False, stop=True)
            sqp = ps.tile([128, QS], F32, tag="w", name="sqp")
            nc.tensor.matmul(sqp, ones_c, zsq2[:, 0:512], start=True, stop=False)
            nc.tensor.matmul(sqp, ones_c, zsq2[:, 512:1024], start=False, stop=True)

            s2 = sb.tile([1, QS], F32)
            nc.scalar.square(s2, szp[0:1, :])
            var = sb.tile([1, QS], F32)
            nc.vector.scalar_tensor_tensor(var, sqp[0:1, :], float(C), s2,
                                           op0=ALU.mult, op1=ALU.subtract)
            lnv = sb.tile([1, QS], F32)
            nc.scalar.activation(lnv, var, AF.Ln, bias=eps2v)
            rstd = sb.tile([1, QS], MMDT)
            nc.scalar.activation(rstd, lnv, AF.Exp, scale=-0.5, bias=lnCv)
            neg_mean = sb.tile([1, QS], F32)
            nc.vector.tensor_scalar_mul(neg_mean, szp[0:1, :], -1.0 / C)
            nmrs = sb.tile([1, QS], MMDT)
            nc.vector.tensor_mul(nmrs, neg_mean, rstd)

            rstd_b = ps.tile([128, QS], F32, tag="w", name="rstd_b")
            nc.tensor.matmul(rstd_b, ones_r, rstd, start=True, stop=True)
            nmrs_b = ps.tile([128, QS], F32, tag="w", name="nmrs_b")
            nc.tensor.matmul(nmrs_b, ones_r, nmrs, start=True, stop=True)

            zln2 = sb.tile([128, 1024], MMDT)
            for cb in range(2):
                zt = sb.tile([128, QS], MMDT, name=f"zt{cb}")
                nc.vector.tensor_mul(zt, zs2[:, cb * 512:(cb + 1) * 512], rstd_b)
                nc.vector.tensor_add(zln2[:, cb * 512:(cb + 1) * 512], zt, nmrs_b)

            # MLP + residual
            hs = [sb.tile([128, QS], MMDT, name=f"hs{i}") for i in range(4)]
            for hb in range(4):
                hps = ps.tile([128, QS], F32, tag="w", name=f"hps{hb}")
                nc.tensor.matmul(hps, wsl(w1_t, 0, hb), zln2[:, 0:512],
                                 start=True, stop=False)
                nc.tensor.matmul(hps, wsl(w1_t, 1, hb), zln2[:, 512:1024],
                                 start=False, stop=True)
                nc.scalar.activation(hs[hb], hps, AF.Gelu, bias=b1p[:, hb:hb + 1])

            tps2 = ps.tile([128, 1024], F32, tag="att", bufs=1, name="tps2")
            for cb in range(2):
                for hb in range(4):
                    nc.tensor.matmul(
                        tps2[:, cb * 512:(cb + 1) * 512],
                        w2_t[:, hb * 256 + cb * 128:hb * 256 + (cb + 1) * 128],
                        hs[hb], start=(hb == 0), stop=(hb == 3))
            ot = [sb.tile([128, QS], F32, name=f"ot{i}") for i in range(2)]
            for cb in range(2):
                nc.vector.scalar_tensor_tensor(
                    ot[cb], tps2[:, cb * 512:(cb + 1) * 512], bvec[cb][:, 2:3],
                    xf[cb], op0=ALU.add, op1=ALU.add)
            # spread the output store over four DMA queues
            nc.sync.dma_start(d_out[0:128, 0:256], ot[0][:, 0:256])
            nc.scalar.dma_start(d_out[0:128, 256:512], ot[0][:, 256:512])
            nc.gpsimd.dma_start(d_out[128:256, 0:256], ot[1][:, 0:256])
            nc.sync.dma_start(d_out[128:256, 256:512], ot[1][:, 256:512])

    nc.compile()
    return nc


_NC = None


def _get_nc():
    global _NC
    if _NC is None:
        _NC = _build_nc()
    return _NC


def _pack_rows(a, nchunk):
    """(nchunk*128, W) -> (128, nchunk*W) with row-chunks side by side."""
    w = a.shape[1]
    out = np.empty((128, nchunk * w), a.dtype)
    for i in range(nchunk):
        out[:, i * w:(i + 1) * w] = a[i * 128:(i + 1) * 128, :]
    return out


def prep_in_maps(x, y, Wq, bq, Wk, bk, Wv, bv, Wo, bo, ln_w, ln_b, W1, b1, W2, b2):
    f = lambda a: np.asarray(a, dtype=np.float32)
    x, y = f(x), f(y)
    Wq, bq, Wk, Wv, bv, Wo, bo = f(Wq), f(bq), f(Wk), f(Wv), f(bv), f(Wo), f(bo)
    ln_w, ln_b, W1, b1, W2, b2 = f(ln_w), f(ln_b), f(W1), f(b1), f(W2), f(b2)

    mmnp = mybir.dt.np(MMDT)
    g = lambda a: np.ascontiguousarray(a).astype(mmnp)

    x_cm = np.ascontiguousarray(x.reshape(C, HW))
    y_cm = np.ascontiguousarray(y.reshape(C, NCTX))

    # host-side algebraic folds
    bo_p = (Wo.astype(np.float64) @ bv.astype(np.float64) + bo).astype(np.float32)
    b1_p = (W1.astype(np.float64) @ ln_b.astype(np.float64) + b1).astype(np.float32)
    W1p = (W1 * ln_w[None, :]).astype(np.float32)

    bvec = np.stack([bq / 16.0, bo_p, b2], axis=1).astype(np.float32)  # (256,3)

    common = {
        "y_mm": g(_pack_rows(y_cm, 2)),
        "wq_mm": g(_pack_rows(Wq.T, 2)),
        "wk_mm": g(_pack_rows(Wk.T, 2)),
        "wv_mm": g(_pack_rows(Wv.T, 2)),
        "wo_mm": g(_pack_rows(Wo.T, 2)),
        "w1_mm": g(_pack_rows(W1p.T, 2)),
        "w2_mm": g(_pack_rows(W2.T, 4)),
        "bvec": bvec,
        "b1p": np.ascontiguousarray(b1_p.reshape(4, 128).T),
        "ones_c": np.ones((128, 128), mmnp),
        "ones_r": np.ones((1, 128), mmnp),
    }
    in_maps = []
    for i in range(NCORES):
        m = dict(common)
        xs = np.ascontiguousarray(x_cm[:, i * QS:(i + 1) * QS])
        m["x_f32"] = xs
        m["x_mm"] = g(_pack_rows(xs, 2))
        in_maps.append(m)
    return in_maps


def kernel(**inputs):
    in_maps = prep_in_maps(**inputs)
    nc = _get_nc()
    res = bass_utils.run_bass_kernel_spmd(nc, in_maps, core_ids=list(range(NCORES)))
    t = np.concatenate([res.results[i]["out_sh"] for i in range(NCORES)], axis=1)
    return t.reshape(1, C, 64, 64)


# revision 7
# speedup vs baseline: 1.2509x; 1.0151x over previous
"""Trainium2 Bass kernel for nn_CMEncoder (cross-attention + LayerNorm2d + MLP block).

Strategy (8 NeuronCores, sequence-parallel over the HW=4096 query tokens):
  - Each core owns 512 query tokens; K/V over the full 4096-token context are
    computed redundantly on every core (no collectives needed).
  - Everything stays channel-major on chip ([feature partition, token free]).
  - Scores are computed transposed (S^T[n, q]) so P = exp(S^T) is the moving
    operand of the P@V matmuls (att^T = V^T @ P); the softmax denominator
    comes from a cheap ones-stationary reduction matmul.
  - All matmuls run in bf16 (2.4 GHz streaming + FWL weight loads, vs 1.2 GHz
    and no FWL for fp32) with fp32 PSUM accumulation.
  - The attention loop is software-pipelined 3 deep so the softmax-exp (ACT)
    latency never blocks the PE's static instruction order.
  - PSUM->SBUF evacuations are split between the scalar and vector engines.
  - Host-side algebraic folds: bk dropped (softmax shift invariance), bv folded
    into the output-projection bias, the 1/sqrt(C) scale folded into the Q
    bias/scale, LayerNorm's affine folded into the MLP's first layer.
  - Only two act-table loads: ln/exp at start, gelu right before the MLP
    (auto-inserted, hidden behind the W1 matmuls).
"""

import math
import numpy as np
import concourse.bacc as bacc
import concourse.mybir as mybir
import concourse.tile as tile
from concourse import bass_utils
from concourse.hw_specs import get_activation_tables

F32 = mybir.dt.float32
BF16 = mybir.dt.bfloat16
AF = mybir.ActivationFunctionType
ALU = mybir.AluOpType

MMDT = BF16      # matmul operand dtype

C = 256          # channels
HW = 4096        # query tokens (64x64)
NCTX = 4096      # context tokens
HID = 512        # mlp hidden
NCORES = 8
QS = HW // NCORES   # 512 queries per core
NBLK = NCTX // 128  # 32 context chunks
EPS = 1e-6


def _build_nc():
    nc = bacc.Bacc("TRN2", target_bir_lowering=False)

    # --- DRAM I/O (weights pre-packed on host: row-chunks side by side) ---
    d_xmm = nc.dram_tensor("x_mm", (128, 2 * QS), MMDT, kind="ExternalInput")
    d_xf = nc.dram_tensor("x_f32", (C, QS), F32, kind="ExternalInput")
    d_y = nc.dram_tensor("y_mm", (128, 2 * NCTX), MMDT, kind="ExternalInput")
    d_wq = nc.dram_tensor("wq_mm", (128, 2 * C), MMDT, kind="ExternalInput")
    d_wk = nc.dram_tensor("wk_mm", (128, 2 * C), MMDT, kind="ExternalInput")
    d_wv = nc.dram_tensor("wv_mm", (128, 2 * C), MMDT, kind="ExternalInput")
    d_wo = nc.dram_tensor("wo_mm", (128, 2 * C), MMDT, kind="ExternalInput")
    d_w1 = nc.dram_tensor("w1_mm", (128, 2 * HID), MMDT, kind="ExternalInput")
    d_w2 = nc.dram_tensor("w2_mm", (128, 4 * C), MMDT, kind="ExternalInput")
    d_bv = nc.dram_tensor("bvec", (C, 3), F32, kind="ExternalInput")   # [bq/16, bo', b2]
    d_b1 = nc.dram_tensor("b1p", (128, 4), F32, kind="ExternalInput")
    d_oc = nc.dram_tensor("ones_c", (128, 128), MMDT, kind="ExternalInput")
    d_or = nc.dram_tensor("ones_r", (1, 128), MMDT, kind="ExternalInput")
    d_out = nc.dram_tensor("out_sh", (C, QS), F32, kind="ExternalOutput")

    tabs = list(get_activation_tables(nc.m.arch).keys())
    LNEXP_SET = tabs.index("natural_log_exp_and_others")

    with tile.TileContext(nc) as tc:
        # Pre-load the exp+ln activation table once; the gelu set is
        # auto-loaded right before the MLP's gelu (the only other set used).
        nc.scalar.add_instruction(mybir.InstLoadActFuncSet(
            name=nc.get_next_instruction_name(), ins=[], outs=[],
            act_func_set_id=LNEXP_SET))

        with (
            tc.tile_pool(name="sb", bufs=1) as sb,
            tc.tile_pool(name="pt_pool", bufs=4) as ptp,
            tc.tile_pool(name="ps", bufs=4, space="PSUM") as ps,
        ):
            # ---------------- input DMAs ----------------
            # sync queue: the tensors the PE needs first, in need-order.
            wq_t = sb.tile([128, 2 * C], MMDT)
            nc.sync.dma_start(wq_t, d_wq[:, :])
            xmm = sb.tile([128, 2 * QS], MMDT)
            nc.sync.dma_start(xmm[:, 0:QS], d_xmm[:, 0:QS])
            nc.sync.dma_start(xmm[:, QS:2 * QS], d_xmm[:, QS:2 * QS])
            # y in ctx-quarters: half 0 on the sync queue, half 1 on scalar
            yq = [[None] * 4 for _ in range(2)]
            for q in range(4):
                for i in range(2):
                    yq[i][q] = sb.tile([128, 1024], MMDT, name=f"y{i}{q}")
                    eng = nc.sync if i == 0 else nc.scalar
                    eng.dma_start(
                        yq[i][q], d_y[:, i * NCTX + q * 1024:i * NCTX + (q + 1) * 1024])

            # gpsimd queue: everything needed later.
            wk_t = sb.tile([128, 2 * C], MMDT)
            nc.gpsimd.dma_start(wk_t, d_wk[:, :])
            wv_t = sb.tile([128, 2 * C], MMDT)
            nc.gpsimd.dma_start(wv_t, d_wv[:, :])
            ones_c = sb.tile([128, 128], MMDT)
            nc.gpsimd.dma_start(ones_c, d_oc[:, :])
            ones_r = sb.tile([1, 128], MMDT)
            nc.gpsimd.dma_start(ones_r, d_or[:, :])
            bvec = [sb.tile([128, 3], F32, name=f"bvec{i}") for i in range(2)]
            for i in range(2):
                nc.gpsimd.dma_start(bvec[i], d_bv[i * 128:(i + 1) * 128, :])
            wo_t = sb.tile([128, 2 * C], MMDT)
            nc.gpsimd.dma_start(wo_t, d_wo[:, :])
            w1_t = sb.tile([128, 2 * HID], MMDT)
            nc.gpsimd.dma_start(w1_t, d_w1[:, :])
            w2_t = sb.tile([128, 4 * C], MMDT)
            nc.gpsimd.dma_start(w2_t, d_w2[:, :])
            b1p = sb.tile([128, 4], F32)
            nc.gpsimd.dma_start(b1p, d_b1[:, :])
            xf = [sb.tile([128, QS], F32, name=f"xf{i}") for i in range(2)]
            for i in range(2):
                nc.gpsimd.dma_start(xf[i], d_xf[i * 128:(i + 1) * 128, :])

            # zero tiles for HAM-warming dummy matmuls
            z128 = sb.tile([128, 128], MMDT)
            nc.vector.memset(z128, 0.0)
            zmv = sb.tile([128, 512], MMDT)
            nc.vector.memset(zmv, 0.0)
            dps = ps.tile([128, 512], F32, tag="dummy", bufs=1, name="dps")

            def warm(mv, n):
                for _ in range(n):
                    nc.tensor.matmul(dps, z128, mv, start=True, stop=True)

            eps2v = sb.tile([1, 1], F32)
            nc.vector.memset(eps2v, float(C) * float(C) * EPS)
            lnCv = sb.tile([1, 1], F32)
            nc.vector.memset(lnCv, math.log(float(C)))

            def wsl(t, cc, cb, w=128):
                # packed weight tile slice: row-chunk cc, col-chunk cb
                return t[:, cc * (t.shape[1] // 2) + cb * w:
                         cc * (t.shape[1] // 2) + (cb + 1) * w]

            def yslice(i, c0, w):
                # y channel-half i, ctx cols [c0, c0+w) (must stay in a quarter)
                q, o = c0 // 1024, c0 % 1024
                return yq[i][q][:, o:o + w]

            # pre-warm the PE clock while input DMAs land
            warm(zmv, 14)

            # ---------------- Q' = (x^T Wq^T + bq)/16, channel-major ----------------
            qp2 = sb.tile([128, 2 * QS], MMDT)
            for cb in range(2):
                qps = ps.tile([128, 512], F32, tag="w", name=f"qps{cb}")
                nc.tensor.matmul(qps, wsl(wq_t, 0, cb), xmm[:, 0:QS],
                                 start=True, stop=False)
                nc.tensor.matmul(qps, wsl(wq_t, 1, cb), xmm[:, QS:2 * QS],
                                 start=False, stop=True)
                nc.scalar.activation(qp2[:, cb * 512:(cb + 1) * 512], qps,
                                     AF.Identity, bias=bvec[cb][:, 0:1],
                                     scale=1.0 / 16.0)

            # ---------------- K^T (channel-major) ----------------
            kt = [sb.tile([128, NCTX], MMDT, name=f"kt{i}") for i in range(2)]
            ev = 0  # evacuation engine round-robin
            for hh in range(4):
                for cb in range(2):
                    c0 = hh * 1024
                    for h in range(2):
                        kps = ps.tile([128, 512], F32, tag="w", name=f"kps{hh}{cb}{h}")
                        nc.tensor.matmul(kps, wsl(wk_t, 0, cb),
                                         yslice(0, c0 + h * 512, 512),
                                         start=True, stop=False)
                        nc.tensor.matmul(kps, wsl(wk_t, 1, cb),
                                         yslice(1, c0 + h * 512, 512),
                                         start=False, stop=True)
                        dst = kt[cb][:, c0 + h * 512:c0 + (h + 1) * 512]
                        if ev % 2 == 0:
                            nc.scalar.copy(dst, kps)
                        else:
                            nc.vector.tensor_copy(dst, kps)
                        ev += 1

            # ---------------- V (token-major) ----------------
            v_all = sb.tile([128, NBLK * 256], MMDT)
            for g in range(16):  # each g covers 2 ctx chunks
                vps = ps.tile([128, 512], F32, tag="w", name=f"vps{g}")
                for k in range(2):
                    ci = g * 2 + k
                    for i in range(2):
                        nc.tensor.matmul(vps[:, k * 256:(k + 1) * 256],
                                         yslice(i, ci * 128, 128),
                                         wv_t[:, i * 256:(i + 1) * 256],
                                         start=(i == 0), stop=(i == 1))
                dst = v_all[:, g * 512:(g + 1) * 512]
                if ev % 2 == 0:
                    nc.scalar.copy(dst, vps)
                else:
                    nc.vector.tensor_copy(dst, vps)
                ev += 1

            # ---------------- attention ----------------
            att2 = ps.tile([128, 2 * QS], F32, tag="att", bufs=1, name="att2")
            csum = ps.tile([128, QS], F32, tag="csum", bufs=1, name="csum")

            def attn_score(i):
                """S^T and exp for context chunk i"""
                sps = ps.tile([128, QS], F32, tag="w", name=f"sps{i}")
                nc.tensor.matmul(sps, kt[0][:, i * 128:(i + 1) * 128],
                                 qp2[:, 0:512], start=True, stop=False)
                nc.tensor.matmul(sps, kt[1][:, i * 128:(i + 1) * 128],
                                 qp2[:, 512:1024], start=False, stop=True)
                pt = ptp.tile([128, QS], MMDT, tag="pt", name=f"pt{i}")
                nc.scalar.activation(pt, sps, AF.Exp)
                return pt

            def attn_accum(i, pt):
                """P@V and colsum accumulation for chunk i"""
                first, last = (i == 0), (i == NBLK - 1)
                for cb in range(2):
                    nc.tensor.matmul(
                        att2[:, cb * 512:(cb + 1) * 512],
                        v_all[:, i * 256 + cb * 128:i * 256 + (cb + 1) * 128],
                        pt, start=first, stop=last)
                nc.tensor.matmul(csum, ones_c, pt, start=first, stop=last)

            # ---- software-pipelined 3 deep: the exp of chunk i completes
            # ---- while the PE runs chunks i+1 / i+2
            p0 = attn_score(0)
            p1 = attn_score(1)
            for i in range(2, NBLK):
                p2 = attn_score(i)
                attn_accum(i - 2, p0)
                p0, p1 = p1, p2
            attn_accum(NBLK - 2, p0)
            attn_accum(NBLK - 1, p1)
            prev_pt = p1

            # evacuate the UN-normalized attention (both copies on the vector
            # engine) while the scalar engine computes 1/colsum via exp(-ln(x));
            # the softmax normalization is applied per-column after Wo instead
            # (z_q = Wo @ attT_q * (1/d_q) + bo commutes with the column scale).
            attnT2 = sb.tile([128, 1024], MMDT)
            for cb in range(2):
                nc.vector.tensor_copy(attnT2[:, cb * 512:(cb + 1) * 512],
                                      att2[:, cb * 512:(cb + 1) * 512])
            lncs = sb.tile([1, QS], F32)
            nc.scalar.activation(lncs, csum[0:1, :], AF.Ln)
            rr = sb.tile([1, QS], MMDT)
            nc.scalar.activation(rr, lncs, AF.Exp, scale=-1.0)
            # keep the PE clock warm while the recip/evac chain runs
            warm(prev_pt, 4)

            # z = Wo @ attT_unnorm
            zps = ps.tile([128, 1024], F32, tag="att", bufs=1, name="zps")
            for cb in range(2):
                nc.tensor.matmul(zps[:, cb * 512:(cb + 1) * 512],
                                 wsl(wo_t, 0, cb), attnT2[:, 0:512],
                                 start=True, stop=False)
                nc.tensor.matmul(zps[:, cb * 512:(cb + 1) * 512],
                                 wsl(wo_t, 1, cb), attnT2[:, 512:1024],
                                 start=False, stop=True)
            rb = ps.tile([128, QS], F32, tag="w", name="rb")
            nc.tensor.matmul(rb, ones_r, rr, start=True, stop=True)
            rbs = sb.tile([128, QS], MMDT)
            nc.vector.tensor_copy(rbs, rb)
            warm(attnT2[:, 0:512], 6)
            zs2 = sb.tile([128, 1024], MMDT)
            zt2m = sb.tile([128, 1024], MMDT)
            for cb in range(2):
                nc.vector.tensor_mul(zt2m[:, cb * 512:(cb + 1) * 512],
                                     zps[:, cb * 512:(cb + 1) * 512], rbs)
                nc.scalar.activation(zs2[:, cb * 512:(cb + 1) * 512],
                                     zt2m[:, cb * 512:(cb + 1) * 512], AF.Identity,
                                     bias=bvec[cb][:, 1:2])
            zsq2 = sb.tile([128, 1024], MMDT)
            nc.vector.tensor_mul(zsq2, zs2, zs2)

            szp = ps.tile([128, QS], F32, tag="csum", bufs=1, name="szp")
            nc.tensor.matmul(szp, ones_c, zs2[:, 0:512], start=True, stop=False)
            nc.tensor.matmul(szp, ones_c, zs2[:, 512:1024], start=False, stop=True)
            sqp = ps.tile([128, QS], F32, tag="w", name="sqp")
            nc.tensor.matmul(sqp, ones_c, zsq2[:, 0:512], start=True, stop=False)
            nc.tensor.matmul(sqp, ones_c, zsq2[:, 512:1024], start=False, stop=True)

            warm(zsq2[:, 0:512], 8)
            s2 = sb.tile([1, QS], F32)
            nc.scalar.square(s2, szp[0:1, :])
            var = sb.tile([1, QS], F32)
            nc.vector.scalar_tensor_tensor(var, sqp[0:1, :], float(C), s2,
                                           op0=ALU.mult, op1=ALU.subtract)
            lnv = sb.tile([1, QS], F32)
            nc.scalar.activation(lnv, var, AF.Ln, bias=eps2v)
            rstd = sb.tile([1, QS], MMDT)
            nc.scalar.activation(rstd, lnv, AF.Exp, scale=-0.5, bias=lnCv)
            neg_mean = sb.tile([1, QS], F32)
            nc.vector.tensor_scalar_mul(neg_mean, szp[0:1, :], -1.0 / C)
            nmrs = sb.tile([1, QS], MMDT)
            nc.vector.tensor_mul(nmrs, neg_mean, rstd)

            rstd_b = ps.tile([128, QS], F32, tag="w", name="rstd_b")
            nc.tensor.matmul(rstd_b, ones_r, rstd, start=True, stop=True)
            nmrs_b = ps.tile([128, QS], F32, tag="w", name="nmrs_b")
            nc.tensor.matmul(nmrs_b, ones_r, nmrs, start=True, stop=True)

            warm(zsq2[:, 512:1024], 6)
            zln2 = sb.tile([128, 1024], MMDT)
            for cb in range(2):
                zt = sb.tile([128, QS], MMDT, name=f"zt{cb}")
                nc.vector.tensor_mul(zt, zs2[:, cb * 512:(cb + 1) * 512], rstd_b)
                nc.vector.tensor_add(zln2[:, cb * 512:(cb + 1) * 512], zt, nmrs_b)

            # MLP + residual
            hs = [sb.tile([128, QS], MMDT, name=f"hs{i}") for i in range(4)]
            for hb in range(4):
                hps = ps.tile([128, QS], F32, tag="w", name=f"hps{hb}")
                nc.tensor.matmul(hps, wsl(w1_t, 0, hb), zln2[:, 0:512],
                                 start=True, stop=False)
                nc.tensor.matmul(hps, wsl(w1_t, 1, hb), zln2[:, 512:1024],
                                 start=False, stop=True)
                nc.scalar.activation(hs[hb], hps, AF.Gelu, bias=b1p[:, hb:hb + 1])

            tps2 = ps.tile([128, 1024], F32, tag="att", bufs=1, name="tps2")
            for cb in range(2):
                for hb in range(4):
                    nc.tensor.matmul(
                        tps2[:, cb * 512:(cb + 1) * 512],
                        w2_t[:, hb * 256 + cb * 128:hb * 256 + (cb + 1) * 128],
                        hs[hb], start=(hb == 0), stop=(hb == 3))
            ot = [sb.tile([128, QS], F32, name=f"ot{i}") for i in range(2)]
            for cb in range(2):
                nc.vector.scalar_tensor_tensor(
                    ot[cb], tps2[:, cb * 512:(cb + 1) * 512], bvec[cb][:, 2:3],
                    xf[cb], op0=ALU.add, op1=ALU.add)
            # spread the output store over four DMA queues
            nc.sync.dma_start(d_out[0:128, 0:256], ot[0][:, 0:256])
            nc.scalar.dma_start(d_out[0:128, 256:512], ot[0][:, 256:512])
            nc.gpsimd.dma_start(d_out[128:256, 0:256], ot[1][:, 0:256])
            nc.sync.dma_start(d_out[128:256, 256:512], ot[1][:, 256:512])

    nc.compile()
    return nc


_NC = None


def _get_nc():
    global _NC
    if _NC is None:
        _NC = _build_nc()
    return _NC


def _pack_rows(a, nchunk):
    """(nchunk*128, W) -> (128, nchunk*W) with row-chunks side by side."""
    w = a.shape[1]
    out = np.empty((128, nchunk * w), a.dtype)
    for i in range(nchunk):
        out[:, i * w:(i + 1) * w] = a[i * 128:(i + 1) * 128, :]
    return out


def prep_in_maps(x, y, Wq, bq, Wk, bk, Wv, bv, Wo, bo, ln_w, ln_b, W1, b1, W2, b2):
    f = lambda a: np.asarray(a, dtype=np.float32)
    x, y = f(x), f(y)
    Wq, bq, Wk, Wv, bv, Wo, bo = f(Wq), f(bq), f(Wk), f(Wv), f(bv), f(Wo), f(bo)
    ln_w, ln_b, W1, b1, W2, b2 = f(ln_w), f(ln_b), f(W1), f(b1), f(W2), f(b2)

    mmnp = mybir.dt.np(MMDT)
    g = lambda a: np.ascontiguousarray(a).astype(mmnp)

    x_cm = np.ascontiguousarray(x.reshape(C, HW))
    y_cm = np.ascontiguousarray(y.reshape(C, NCTX))

    # host-side algebraic folds
    bo_p = (Wo.astype(np.float64) @ bv.astype(np.float64) + bo).astype(np.float32)
    b1_p = (W1.astype(np.float64) @ ln_b.astype(np.float64) + b1).astype(np.float32)
    W1p = (W1 * ln_w[None, :]).astype(np.float32)

    bvec = np.stack([bq / 16.0, bo_p, b2], axis=1).astype(np.float32)  # (256,3)

    common = {
        "y_mm": g(_pack_rows(y_cm, 2)),
        "wq_mm": g(_pack_rows(Wq.T, 2)),
        "wk_mm": g(_pack_rows(Wk.T, 2)),
        "wv_mm": g(_pack_rows(Wv.T, 2)),
        "wo_mm": g(_pack_rows(Wo.T, 2)),
        "w1_mm": g(_pack_rows(W1p.T, 2)),
        "w2_mm": g(_pack_rows(W2.T, 4)),
        "bvec": bvec,
        "b1p": np.ascontiguousarray(b1_p.reshape(4, 128).T),
        "ones_c": np.ones((128, 128), mmnp),
        "ones_r": np.ones((1, 128), mmnp),
    }
    in_maps = []
    for i in range(NCORES):
        m = dict(common)
        xs = np.ascontiguousarray(x_cm[:, i * QS:(i + 1) * QS])
        m["x_f32"] = xs
        m["x_mm"] = g(_pack_rows(xs, 2))
        in_maps.append(m)
    return in_maps


def kernel(**inputs):
    in_maps = prep_in_maps(**inputs)
    nc = _get_nc()
    res = bass_utils.run_bass_kernel_spmd(nc, in_maps, core_ids=list(range(NCORES)))
    t = np.concatenate([res.results[i]["out_sh"] for i in range(NCORES)], axis=1)
    return t.reshape(1, C, 64, 64)


# revision 8
# speedup vs baseline: 1.2758x; 1.0199x over previous
"""Trainium2 Bass kernel for nn_CMEncoder (cross-attention + LayerNorm2d + MLP block).

Strategy (8 NeuronCores, sequence-parallel over the HW=4096 query tokens):
  - Each core owns 512 query tokens; K/V over the full 4096-token context are
    computed redundantly on every core (no collectives needed).
  - Everything stays channel-major on chip ([feature partition, token free]).
  - Scores are computed transposed (S^T[n, q]) so P = exp(S^T) is the moving
    operand of the P@V matmuls (att^T = V^T @ P); the softmax denominator
    comes from a cheap ones-stationary reduction matmul.
  - All matmuls run in bf16 (2.4 GHz streaming + FWL weight loads, vs 1.2 GHz
    and no FWL for fp32) with fp32 PSUM accumulation.
  - The attention loop is software-pipelined 3 deep so the softmax-exp (ACT)
    latency never blocks the PE's static instruction order.
  - PSUM->SBUF evacuations are split between the scalar and vector engines.
  - Host-side algebraic folds: bk dropped (softmax shift invariance), bv folded
    into the output-projection bias, the 1/sqrt(C) scale folded into the Q
    bias/scale, LayerNorm's affine folded into the MLP's first layer.
  - Only two act-table loads: ln/exp at start, gelu right before the MLP
    (auto-inserted, hidden behind the W1 matmuls).
"""

import math
import numpy as np
import concourse.bacc as bacc
import concourse.mybir as mybir
import concourse.tile as tile
from concourse import bass_utils
from concourse.hw_specs import get_activation_tables

F32 = mybir.dt.float32
BF16 = mybir.dt.bfloat16
AF = mybir.ActivationFunctionType
ALU = mybir.AluOpType

MMDT = BF16      # matmul operand dtype

C = 256          # channels
HW = 4096        # query tokens (64x64)
NCTX = 4096      # context tokens
HID = 512        # mlp hidden
NCORES = 8
QS = HW // NCORES   # 512 queries per core
NBLK = NCTX // 128  # 32 context chunks
EPS = 1e-6


def _build_nc():
    nc = bacc.Bacc("TRN2", target_bir_lowering=False)

    # --- DRAM I/O (weights pre-packed on host: row-chunks side by side) ---
    d_xmm = nc.dram_tensor("x_mm", (128, 2 * QS), MMDT, kind="ExternalInput")
    d_xf = nc.dram_tensor("x_f32", (C, QS), F32, kind="ExternalInput")
    d_y = nc.dram_tensor("y_mm", (128, 2 * NCTX), MMDT, kind="ExternalInput")
    d_wq = nc.dram_tensor("wq_mm", (128, 2 * C), MMDT, kind="ExternalInput")
    d_wk = nc.dram_tensor("wk_mm", (128, 2 * C), MMDT, kind="ExternalInput")
    d_wv = nc.dram_tensor("wv_mm", (128, 2 * C), MMDT, kind="ExternalInput")
    d_wo = nc.dram_tensor("wo_mm", (128, 2 * C), MMDT, kind="ExternalInput")
    d_w1 = nc.dram_tensor("w1_mm", (128, 2 * HID), MMDT, kind="ExternalInput")
    d_w2 = nc.dram_tensor("w2_mm", (128, 4 * C), MMDT, kind="ExternalInput")
    d_bv = nc.dram_tensor("bvec", (C, 3), F32, kind="ExternalInput")   # [bq/16, bo', b2]
    d_b1 = nc.dram_tensor("b1p", (128, 4), F32, kind="ExternalInput")
    d_oc = nc.dram_tensor("ones_c", (128, 128), MMDT, kind="ExternalInput")
    d_or = nc.dram_tensor("ones_r", (1, 128), MMDT, kind="ExternalInput")
    d_out = nc.dram_tensor("out_sh", (C, QS), F32, kind="ExternalOutput")

    tabs = list(get_activation_tables(nc.m.arch).keys())
    LNEXP_SET = tabs.index("natural_log_exp_and_others")

    with tile.TileContext(nc) as tc:
        # Pre-load the exp+ln activation table once; the gelu set is
        # auto-loaded right before the MLP's gelu (the only other set used).
        nc.scalar.add_instruction(mybir.InstLoadActFuncSet(
            name=nc.get_next_instruction_name(), ins=[], outs=[],
            act_func_set_id=LNEXP_SET))

        with (
            tc.tile_pool(name="sb", bufs=1) as sb,
            tc.tile_pool(name="pt_pool", bufs=4) as ptp,
            tc.tile_pool(name="ps", bufs=4, space="PSUM") as ps,
        ):
            # ---------------- input DMAs ----------------
            # sync queue: the tensors the PE needs first, in need-order.
            wq_t = sb.tile([128, 2 * C], MMDT)
            nc.sync.dma_start(wq_t, d_wq[:, :])
            xmm = sb.tile([128, 2 * QS], MMDT)
            nc.sync.dma_start(xmm[:, 0:QS], d_xmm[:, 0:QS])
            nc.sync.dma_start(xmm[:, QS:2 * QS], d_xmm[:, QS:2 * QS])
            # y in ctx-quarters: half 0 on the sync queue, half 1 on scalar
            yq = [[None] * 4 for _ in range(2)]
            for q in range(4):
                for i in range(2):
                    yq[i][q] = sb.tile([128, 1024], MMDT, name=f"y{i}{q}")
                    eng = nc.sync if i == 0 else nc.scalar
                    eng.dma_start(
                        yq[i][q], d_y[:, i * NCTX + q * 1024:i * NCTX + (q + 1) * 1024])

            # gpsimd queue: everything needed later.
            wk_t = sb.tile([128, 2 * C], MMDT)
            nc.gpsimd.dma_start(wk_t, d_wk[:, :])
            wv_t = sb.tile([128, 2 * C], MMDT)
            nc.gpsimd.dma_start(wv_t, d_wv[:, :])
            ones_c = sb.tile([128, 128], MMDT)
            nc.gpsimd.dma_start(ones_c, d_oc[:, :])
            ones_r = sb.tile([1, 128], MMDT)
            nc.gpsimd.dma_start(ones_r, d_or[:, :])
            bvec = [sb.tile([128, 3], F32, name=f"bvec{i}") for i in range(2)]
            for i in range(2):
                nc.gpsimd.dma_start(bvec[i], d_bv[i * 128:(i + 1) * 128, :])
            wo_t = sb.tile([128, 2 * C], MMDT)
            nc.gpsimd.dma_start(wo_t, d_wo[:, :])
            w1_t = sb.tile([128, 2 * HID], MMDT)
            nc.gpsimd.dma_start(w1_t, d_w1[:, :])
            w2_t = sb.tile([128, 4 * C], MMDT)
            nc.gpsimd.dma_start(w2_t, d_w2[:, :])
            b1p = sb.tile([128, 4], F32)
            nc.gpsimd.dma_start(b1p, d_b1[:, :])
            xf = [sb.tile([128, QS], F32, name=f"xf{i}") for i in range(2)]
            for i in range(2):
                nc.gpsimd.dma_start(xf[i], d_xf[i * 128:(i + 1) * 128, :])

            # zero tiles for HAM-warming dummy matmuls
            z128 = sb.tile([128, 128], MMDT)
            nc.vector.memset(z128, 0.0)
            zmv = sb.tile([128, 512], MMDT)
            nc.vector.memset(zmv, 0.0)
            dps = ps.tile([128, 512], F32, tag="dummy", bufs=1, name="dps")

            def warm(mv, n):
                for _ in range(n):
                    nc.tensor.matmul(dps, z128, mv, start=True, stop=True)

            eps2v = sb.tile([1, 1], F32)
            nc.vector.memset(eps2v, float(C) * float(C) * EPS)
            lnCv = sb.tile([1, 1], F32)
            nc.vector.memset(lnCv, math.log(float(C)))

            def wsl(t, cc, cb, w=128):
                # packed weight tile slice: row-chunk cc, col-chunk cb
                return t[:, cc * (t.shape[1] // 2) + cb * w:
                         cc * (t.shape[1] // 2) + (cb + 1) * w]

            def yslice(i, c0, w):
                # y channel-half i, ctx cols [c0, c0+w) (must stay in a quarter)
                q, o = c0 // 1024, c0 % 1024
                return yq[i][q][:, o:o + w]

            # pre-warm the PE clock while input DMAs land
            warm(zmv, 14)

            # ---------------- Q' = (x^T Wq^T + bq)/16, channel-major ----------------
            qp2 = sb.tile([128, 2 * QS], MMDT)
            for cb in range(2):
                qps = ps.tile([128, 512], F32, tag="w", name=f"qps{cb}")
                nc.tensor.matmul(qps, wsl(wq_t, 0, cb), xmm[:, 0:QS],
                                 start=True, stop=False)
                nc.tensor.matmul(qps, wsl(wq_t, 1, cb), xmm[:, QS:2 * QS],
                                 start=False, stop=True)
                nc.scalar.activation(qp2[:, cb * 512:(cb + 1) * 512], qps,
                                     AF.Identity, bias=bvec[cb][:, 0:1],
                                     scale=1.0 / 16.0)

            # ---------------- K^T / V / attention, interleaved per ctx-quarter ----
            # Each y-quarter is turned into K^T and V as soon as its DMA lands,
            # then its 8 attention chunks run; the next quarter's K/V matmuls
            # are emitted inside the score/accum pipeline so the PE never
            # waits on HBM after the first quarter.
            kt = [sb.tile([128, NCTX], MMDT, name=f"kt{i}") for i in range(2)]
            v_all = sb.tile([128, NBLK * 256], MMDT)
            att2 = ps.tile([128, 2 * QS], F32, tag="att", bufs=1, name="att2")
            csum = ps.tile([128, QS], F32, tag="csum", bufs=1, name="csum")
            ev = 0  # evacuation engine round-robin

            def kv_quarter(qq):
                nonlocal ev
                c0 = qq * 1024
                for cb in range(2):
                    for h in range(2):
                        kps = ps.tile([128, 512], F32, tag="w", name=f"kps{qq}{cb}{h}")
                        nc.tensor.matmul(kps, wsl(wk_t, 0, cb),
                                         yslice(0, c0 + h * 512, 512),
                                         start=True, stop=False)
                        nc.tensor.matmul(kps, wsl(wk_t, 1, cb),
                                         yslice(1, c0 + h * 512, 512),
                                         start=False, stop=True)
                        dst = kt[cb][:, c0 + h * 512:c0 + (h + 1) * 512]
                        if ev % 2 == 0:
                            nc.scalar.copy(dst, kps)
                        else:
                            nc.vector.tensor_copy(dst, kps)
                        ev += 1
                for g in range(4):  # each g covers 2 ctx chunks
                    vps = ps.tile([128, 512], F32, tag="w", name=f"vps{qq}{g}")
                    for k in range(2):
                        ci = qq * 8 + g * 2 + k
                        for i in range(2):
                            nc.tensor.matmul(vps[:, k * 256:(k + 1) * 256],
                                             yslice(i, ci * 128, 128),
                                             wv_t[:, i * 256:(i + 1) * 256],
                                             start=(i == 0), stop=(i == 1))
                    dst = v_all[:, (qq * 4 + g) * 512:(qq * 4 + g + 1) * 512]
                    if ev % 2 == 0:
                        nc.scalar.copy(dst, vps)
                    else:
                        nc.vector.tensor_copy(dst, vps)
                    ev += 1

            def attn_score(i):
                """S^T and exp for context chunk i"""
                sps = ps.tile([128, QS], F32, tag="w", name=f"sps{i}")
                nc.tensor.matmul(sps, kt[0][:, i * 128:(i + 1) * 128],
                                 qp2[:, 0:512], start=True, stop=False)
                nc.tensor.matmul(sps, kt[1][:, i * 128:(i + 1) * 128],
                                 qp2[:, 512:1024], start=False, stop=True)
                pt = ptp.tile([128, QS], MMDT, tag="pt", name=f"pt{i}")
                nc.scalar.activation(pt, sps, AF.Exp)
                return pt

            def attn_accum(i, pt):
                """P@V and colsum accumulation for chunk i"""
                first, last = (i == 0), (i == NBLK - 1)
                for cb in range(2):
                    nc.tensor.matmul(
                        att2[:, cb * 512:(cb + 1) * 512],
                        v_all[:, i * 256 + cb * 128:i * 256 + (cb + 1) * 128],
                        pt, start=first, stop=last)
                nc.tensor.matmul(csum, ones_c, pt, start=first, stop=last)

            # quarter 0's K/V, then the 3-deep score/accum pipeline with the
            # next quarter's K/V emitted right before its first score
            kv_quarter(0)
            p0 = attn_score(0)
            p1 = attn_score(1)
            for i in range(2, NBLK):
                if i % 8 == 0:
                    kv_quarter(i // 8)
                p2 = attn_score(i)
                attn_accum(i - 2, p0)
                p0, p1 = p1, p2
            attn_accum(NBLK - 2, p0)
            attn_accum(NBLK - 1, p1)
            prev_pt = p1

            # evacuate the UN-normalized attention (both copies on the vector
            # engine) while the scalar engine computes 1/colsum via exp(-ln(x));
            # the softmax normalization is applied per-column after Wo instead
            # (z_q = Wo @ attT_q * (1/d_q) + bo commutes with the column scale).
            attnT2 = sb.tile([128, 1024], MMDT)
            nc.vector.tensor_copy(attnT2[:, 0:512], att2[:, 0:512])
            nc.vector.tensor_copy(attnT2[:, 512:1024], att2[:, 512:1024])
            lncs = sb.tile([1, QS], F32)
            nc.scalar.activation(lncs, csum[0:1, :], AF.Ln)
            rr = sb.tile([1, QS], MMDT)
            nc.scalar.activation(rr, lncs, AF.Exp, scale=-1.0)
            # keep the PE clock warm while the recip/evac chain runs
            warm(prev_pt, 4)

            # z = Wo @ attT_unnorm
            zps = ps.tile([128, 1024], F32, tag="att", bufs=1, name="zps")
            for cb in range(2):
                nc.tensor.matmul(zps[:, cb * 512:(cb + 1) * 512],
                                 wsl(wo_t, 0, cb), attnT2[:, 0:512],
                                 start=True, stop=False)
                nc.tensor.matmul(zps[:, cb * 512:(cb + 1) * 512],
                                 wsl(wo_t, 1, cb), attnT2[:, 512:1024],
                                 start=False, stop=True)
            rb = ps.tile([128, QS], F32, tag="w", name="rb")
            nc.tensor.matmul(rb, ones_r, rr, start=True, stop=True)
            rbs = sb.tile([128, QS], MMDT)
            nc.scalar.copy(rbs, rb)
            warm(attnT2[:, 0:512], 6)
            zs2 = sb.tile([128, 1024], MMDT)
            zt2m = sb.tile([128, 1024], MMDT)
            for cb in range(2):
                nc.vector.tensor_mul(zt2m[:, cb * 512:(cb + 1) * 512],
                                     zps[:, cb * 512:(cb + 1) * 512], rbs)
                nc.scalar.activation(zs2[:, cb * 512:(cb + 1) * 512],
                                     zt2m[:, cb * 512:(cb + 1) * 512], AF.Identity,
                                     bias=bvec[cb][:, 1:2])
            zsq2 = sb.tile([128, 1024], MMDT)
            nc.scalar.square(zsq2, zs2)

            szp = ps.tile([128, QS], F32, tag="csum", bufs=1, name="szp")
            nc.tensor.matmul(szp, ones_c, zs2[:, 0:512], start=True, stop=False)
            nc.tensor.matmul(szp, ones_c, zs2[:, 512:1024], start=False, stop=True)
            sqp = ps.tile([128, QS], F32, tag="w", name="sqp")
            nc.tensor.matmul(sqp, ones_c, zsq2[:, 0:512], start=True, stop=False)
            nc.tensor.matmul(sqp, ones_c, zsq2[:, 512:1024], start=False, stop=True)

            warm(zsq2[:, 0:512], 8)
            s2 = sb.tile([1, QS], F32)
            nc.scalar.square(s2, szp[0:1, :])
            var = sb.tile([1, QS], F32)
            nc.vector.scalar_tensor_tensor(var, sqp[0:1, :], float(C), s2,
                                           op0=ALU.mult, op1=ALU.subtract)
            lnv = sb.tile([1, QS], F32)
            nc.scalar.activation(lnv, var, AF.Ln, bias=eps2v)
            rstd = sb.tile([1, QS], MMDT)
            nc.scalar.activation(rstd, lnv, AF.Exp, scale=-0.5, bias=lnCv)
            neg_mean = sb.tile([1, QS], F32)
            nc.vector.tensor_scalar_mul(neg_mean, szp[0:1, :], -1.0 / C)
            nmrs = sb.tile([1, QS], MMDT)
            nc.vector.tensor_mul(nmrs, neg_mean, rstd)

            rstd_b = ps.tile([128, QS], F32, tag="w", name="rstd_b")
            nc.tensor.matmul(rstd_b, ones_r, rstd, start=True, stop=True)
            nmrs_b = ps.tile([128, QS], F32, tag="w", name="nmrs_b")
            nc.tensor.matmul(nmrs_b, ones_r, nmrs, start=True, stop=True)

            warm(zsq2[:, 512:1024], 6)
            zln2 = sb.tile([128, 1024], MMDT)
            for cb in range(2):
                zt = sb.tile([128, QS], MMDT, name=f"zt{cb}")
                nc.vector.tensor_mul(zt, zs2[:, cb * 512:(cb + 1) * 512], rstd_b)
                nc.vector.tensor_add(zln2[:, cb * 512:(cb + 1) * 512], zt, nmrs_b)

            # MLP + residual
            hs = [sb.tile([128, QS], MMDT, name=f"hs{i}") for i in range(4)]
            for hb in range(4):
                hps = ps.tile([128, QS], F32, tag="w", name=f"hps{hb}")
                nc.tensor.matmul(hps, wsl(w1_t, 0, hb), zln2[:, 0:512],
                                 start=True, stop=False)
                nc.tensor.matmul(hps, wsl(w1_t, 1, hb), zln2[:, 512:1024],
                                 start=False, stop=True)
                nc.scalar.activation(hs[hb], hps, AF.Gelu, bias=b1p[:, hb:hb + 1])

            tps2 = ps.tile([128, 1024], F32, tag="att", bufs=1, name="tps2")
            for cb in range(2):
                for hb in range(4):
                    nc.tensor.matmul(
                        tps2[:, cb * 512:(cb + 1) * 512],
                        w2_t[:, hb * 256 + cb * 128:hb * 256 + (cb + 1) * 128],
                        hs[hb], start=(hb == 0), stop=(hb == 3))
            ot = [sb.tile([128, QS], F32, name=f"ot{i}") for i in range(2)]
            for cb in range(2):
                nc.vector.scalar_tensor_tensor(
                    ot[cb], tps2[:, cb * 512:(cb + 1) * 512], bvec[cb][:, 2:3],
                    xf[cb], op0=ALU.add, op1=ALU.add)
            # spread the output store over four DMA queues
            nc.sync.dma_start(d_out[0:128, 0:256], ot[0][:, 0:256])
            nc.scalar.dma_start(d_out[0:128, 256:512], ot[0][:, 256:512])
            nc.gpsimd.dma_start(d_out[128:256, 0:256], ot[1][:, 0:256])
            nc.sync.dma_start(d_out[128:256, 256:512], ot[1][:, 256:512])

    nc.compile()
    return nc


_NC = None


def _get_nc():
    global _NC
    if _NC is None:
        _NC = _build_nc()
    return _NC


def _pack_rows(a, nchunk):
    """(nchunk*128, W) -> (128, nchunk*W) with row-chunks side by side."""
    w = a.shape[1]
    out = np.empty((128, nchunk * w), a.dtype)
    for i in range(nchunk):
        out[:, i * w:(i + 1) * w] = a[i * 128:(i + 1) * 128, :]
    return out


def prep_in_maps(x, y, Wq, bq, Wk, bk, Wv, bv, Wo, bo, ln_w, ln_b, W1, b1, W2, b2):
    f = lambda a: np.asarray(a, dtype=np.float32)
    x, y = f(x), f(y)
    Wq, bq, Wk, Wv, bv, Wo, bo = f(Wq), f(bq), f(Wk), f(Wv), f(bv), f(Wo), f(bo)
    ln_w, ln_b, W1, b1, W2, b2 = f(ln_w), f(ln_b), f(W1), f(b1), f(W2), f(b2)

    mmnp = mybir.dt.np(MMDT)
    g = lambda a: np.ascontiguousarray(a).astype(mmnp)

    x_cm = np.ascontiguousarray(x.reshape(C, HW))
    y_cm = np.ascontiguousarray(y.reshape(C, NCTX))

    # host-side algebraic folds
    bo_p = (Wo.astype(np.float64) @ bv.astype(np.float64) + bo).astype(np.float32)
    b1_p = (W1.astype(np.float64) @ ln_b.astype(np.float64) + b1).astype(np.float32)
    W1p = (W1 * ln_w[None, :]).astype(np.float32)

    bvec = np.stack([bq / 16.0, bo_p, b2], axis=1).astype(np.float32)  # (256,3)

    common = {
        "y_mm": g(_pack_rows(y_cm, 2)),
        "wq_mm": g(_pack_rows(Wq.T, 2)),
        "wk_mm": g(_pack_rows(Wk.T, 2)),
        "wv_mm": g(_pack_rows(Wv.T, 2)),
        "wo_mm": g(_pack_rows(Wo.T, 2)),
        "w1_mm": g(_pack_rows(W1p.T, 2)),
        "w2_mm": g(_pack_rows(W2.T, 4)),
        "bvec": bvec,
        "b1p": np.ascontiguousarray(b1_p.reshape(4, 128).T),
        "ones_c": np.ones((128, 128), mmnp),
        "ones_r": np.ones((1, 128), mmnp),
    }
    in_maps = []
    for i in range(NCORES):
        m = dict(common)
        xs = np.ascontiguousarray(x_cm[:, i * QS:(i + 1) * QS])
        m["x_f32"] = xs
        m["x_mm"] = g(_pack_rows(xs, 2))
        in_maps.append(m)
    return in_maps


def kernel(**inputs):
    in_maps = prep_in_maps(**inputs)
    nc = _get_nc()
    res = bass_utils.run_bass_kernel_spmd(nc, in_maps, core_ids=list(range(NCORES)))
    t = np.concatenate([res.results[i]["out_sh"] for i in range(NCORES)], axis=1)
    return t.reshape(1, C, 64, 64)


# revision 10
# speedup vs baseline: 1.3532x; 1.0607x over previous
"""Trainium2 Bass kernel for nn_CMEncoder (cross-attention + LayerNorm2d + MLP block).

Strategy (8 NeuronCores, sequence-parallel over the HW=4096 query tokens):
  - Each core owns 512 query tokens; K/V over the full 4096-token context are
    computed redundantly on every core (no collectives needed).
  - Everything stays channel-major on chip ([feature partition, token free]).
  - Scores are computed transposed (S^T[n, q]) so P = exp(S^T) is the moving
    operand of the P@V matmuls (att^T = V^T @ P); the softmax denominator
    comes from a cheap ones-stationary reduction matmul.
  - All matmuls run in bf16 (2.4 GHz streaming + FWL weight loads, vs 1.2 GHz
    and no FWL for fp32) with fp32 PSUM accumulation.
  - The attention loop is software-pipelined 3 deep so the softmax-exp (ACT)
    latency never blocks the PE's static instruction order.
  - PSUM->SBUF evacuations are split between the scalar and vector engines.
  - Host-side algebraic folds: bk dropped (softmax shift invariance), bv folded
    into the output-projection bias, the 1/sqrt(C) scale folded into the Q
    bias/scale, LayerNorm's affine folded into the MLP's first layer.
  - Only two act-table loads: ln/exp at start, gelu right before the MLP
    (auto-inserted, hidden behind the W1 matmuls).
"""

import math
import numpy as np
import concourse.bacc as bacc
import concourse.mybir as mybir
import concourse.tile as tile
from concourse import bass_utils
from concourse.hw_specs import get_activation_tables

F32 = mybir.dt.float32
BF16 = mybir.dt.bfloat16
FP8 = mybir.dt.float8e4
AF = mybir.ActivationFunctionType
ALU = mybir.AluOpType

MMDT = BF16      # matmul operand dtype

C = 256          # channels
HW = 4096        # query tokens (64x64)
NCTX = 4096      # context tokens
HID = 512        # mlp hidden
NCORES = 8
QS = HW // NCORES   # 512 queries per core
NBLK = NCTX // 128  # 32 context chunks
EPS = 1e-6


def _build_nc():
    nc = bacc.Bacc("TRN2", target_bir_lowering=False)

    # --- DRAM I/O (weights pre-packed on host: row-chunks side by side) ---
    d_xmm = nc.dram_tensor("x_mm", (128, 2 * QS), MMDT, kind="ExternalInput")
    d_xf = nc.dram_tensor("x_f32", (C, QS), F32, kind="ExternalInput")
    d_y = nc.dram_tensor("y_mm", (128, 2 * NCTX), MMDT, kind="ExternalInput")
    d_wq = nc.dram_tensor("wq_mm", (128, 2 * C), MMDT, kind="ExternalInput")
    d_wk = nc.dram_tensor("wk_mm", (128, 2 * C), MMDT, kind="ExternalInput")
    d_wv = nc.dram_tensor("wv_mm", (128, 2 * C), MMDT, kind="ExternalInput")
    d_wo = nc.dram_tensor("wo_mm", (128, 2 * C), MMDT, kind="ExternalInput")
    d_w1 = nc.dram_tensor("w1_mm", (128, 2 * HID), MMDT, kind="ExternalInput")
    d_w2 = nc.dram_tensor("w2_mm", (128, 4 * C), MMDT, kind="ExternalInput")
    d_bv = nc.dram_tensor("bvec", (C, 3), F32, kind="ExternalInput")   # [bq/16, bo', b2]
    d_b1 = nc.dram_tensor("b1p", (128, 4), F32, kind="ExternalInput")
    d_oc = nc.dram_tensor("ones_c", (128, 256), FP8, kind="ExternalInput")
    d_ocb = nc.dram_tensor("ones_cb", (128, 128), MMDT, kind="ExternalInput")
    d_or = nc.dram_tensor("ones_r", (1, 128), MMDT, kind="ExternalInput")
    d_out = nc.dram_tensor("out_sh", (C, QS), F32, kind="ExternalOutput")

    tabs = list(get_activation_tables(nc.m.arch).keys())
    LNEXP_SET = tabs.index("natural_log_exp_and_others")

    with tile.TileContext(nc) as tc:
        # Pre-load the exp+ln activation table once; the gelu set is
        # auto-loaded right before the MLP's gelu (the only other set used).
        nc.scalar.add_instruction(mybir.InstLoadActFuncSet(
            name=nc.get_next_instruction_name(), ins=[], outs=[],
            act_func_set_id=LNEXP_SET))

        with (
            tc.tile_pool(name="sb", bufs=1) as sb,
            tc.tile_pool(name="pt_pool", bufs=4) as ptp,
            tc.tile_pool(name="ps", bufs=2, space="PSUM") as ps,
        ):
            # ---------------- input DMAs ----------------
            # sync queue: the tensors the PE needs first, in need-order.
            wq_t = sb.tile([128, 2 * C], MMDT)
            nc.sync.dma_start(wq_t, d_wq[:, :])
            xmm = sb.tile([128, 2 * QS], MMDT)
            nc.sync.dma_start(xmm[:, 0:QS], d_xmm[:, 0:QS])
            nc.sync.dma_start(xmm[:, QS:2 * QS], d_xmm[:, QS:2 * QS])
            # y in ctx-quarters: half 0 on the sync queue, half 1 on scalar
            yq = [[None] * 4 for _ in range(2)]
            for q in range(4):
                for i in range(2):
                    yq[i][q] = sb.tile([128, 1024], MMDT, name=f"y{i}{q}")
                    eng = nc.sync if i == 0 else nc.scalar
                    eng.dma_start(
                        yq[i][q], d_y[:, i * NCTX + q * 1024:i * NCTX + (q + 1) * 1024])

            # gpsimd queue: everything needed later.
            wk_t = sb.tile([128, 2 * C], MMDT)
            nc.gpsimd.dma_start(wk_t, d_wk[:, :])
            wv_t = sb.tile([128, 2 * C], MMDT)
            nc.gpsimd.dma_start(wv_t, d_wv[:, :])
            ones3 = sb.tile([128, 2, 128], FP8)
            nc.gpsimd.dma_start(ones3, d_oc[:, :])
            ones_cb = sb.tile([128, 128], MMDT)
            nc.gpsimd.dma_start(ones_cb, d_ocb[:, :])
            ones_r = sb.tile([1, 128], MMDT)
            nc.gpsimd.dma_start(ones_r, d_or[:, :])
            bvec = [sb.tile([128, 3], F32, name=f"bvec{i}") for i in range(2)]
            for i in range(2):
                nc.gpsimd.dma_start(bvec[i], d_bv[i * 128:(i + 1) * 128, :])
            wo_t = sb.tile([128, 2 * C], MMDT)
            nc.gpsimd.dma_start(wo_t, d_wo[:, :])
            w1_t = sb.tile([128, 2 * HID], MMDT)
            nc.gpsimd.dma_start(w1_t, d_w1[:, :])
            w2_t = sb.tile([128, 4 * C], MMDT)
            nc.gpsimd.dma_start(w2_t, d_w2[:, :])
            b1p = sb.tile([128, 4], F32)
            nc.gpsimd.dma_start(b1p, d_b1[:, :])
            xf = [sb.tile([128, QS], F32, name=f"xf{i}") for i in range(2)]
            for i in range(2):
                nc.gpsimd.dma_start(xf[i], d_xf[i * 128:(i + 1) * 128, :])

            # zero tiles for HAM-warming dummy matmuls
            z128 = sb.tile([128, 128], MMDT)
            nc.vector.memset(z128, 0.0)
            zmv = sb.tile([128, 512], MMDT)
            nc.vector.memset(zmv, 0.0)
            dps = ps.tile([128, 512], F32, tag="dummy", bufs=1, name="dps")

            def warm(mv, n):
                for _ in range(n):
                    nc.tensor.matmul(dps, z128, mv, start=True, stop=True)

            eps2v = sb.tile([1, 1], F32)
            nc.vector.memset(eps2v, float(C) * float(C) * EPS)
            lnCv = sb.tile([1, 1], F32)
            nc.vector.memset(lnCv, math.log(float(C)))

            def wsl(t, cc, cb, w=128):
                # packed weight tile slice: row-chunk cc, col-chunk cb
                return t[:, cc * (t.shape[1] // 2) + cb * w:
                         cc * (t.shape[1] // 2) + (cb + 1) * w]

            def yslice(i, c0, w):
                # y channel-half i, ctx cols [c0, c0+w) (must stay in a quarter)
                q, o = c0 // 1024, c0 % 1024
                return yq[i][q][:, o:o + w]

            # pre-warm the PE clock while input DMAs land
            warm(zmv, 14)

            # ---------------- Q' = (x^T Wq^T + bq)/16, channel-major, fp8 ---------
            qp3 = sb.tile([128, 2, QS], FP8)
            for cb in range(2):
                qps = ps.tile([128, 1024], F32, tag="w", name=f"qps{cb}")
                nc.tensor.matmul(qps[:, 0:512], wsl(wq_t, 0, cb), xmm[:, 0:QS],
                                 start=True, stop=False)
                nc.tensor.matmul(qps[:, 0:512], wsl(wq_t, 1, cb), xmm[:, QS:2 * QS],
                                 start=False, stop=True)
                nc.scalar.activation(qp3[:, cb, :], qps[:, 0:512],
                                     AF.Identity, bias=bvec[cb][:, 0:1],
                                     scale=1.0 / 16.0)

            # ------- K^T / V / attention (fp8 DoubleRow), interleaved per quarter --
            # kt3[c_half, chan, ctx] and v3[ctx128, chunk, chan] are stored fp8
            # with the 256-deep contraction split across the DoubleRow plane
            # dim, so each score needs ONE matmul and each P@V pair ONE matmul
            # per output half (2 fp8 weights/cell, 2 multiplies/cycle).
            kt3 = sb.tile([128, 2, NCTX], FP8)
            v3 = sb.tile([128, NBLK, 256], FP8)
            att2 = ps.tile([128, 2 * QS], F32, tag="att", bufs=1, name="att2")
            csum = ps.tile([128, QS], F32, tag="csum", bufs=1, name="csum")
            ev = 0  # evacuation engine round-robin

            def kv_quarter(qq):
                nonlocal ev
                c0 = qq * 1024
                for cb in range(2):
                    kps = ps.tile([128, 1024], F32, tag="w", name=f"kps{qq}{cb}")
                    for h in range(2):
                        nc.tensor.matmul(kps[:, h * 512:(h + 1) * 512],
                                         wsl(wk_t, 0, cb),
                                         yslice(0, c0 + h * 512, 512),
                                         start=True, stop=False)
                        nc.tensor.matmul(kps[:, h * 512:(h + 1) * 512],
                                         wsl(wk_t, 1, cb),
                                         yslice(1, c0 + h * 512, 512),
                                         start=False, stop=True)
                    dst = kt3[:, cb, c0:c0 + 1024]
                    if ev % 2 == 0:
                        nc.scalar.copy(dst, kps)
                    else:
                        nc.vector.tensor_copy(dst, kps)
                    ev += 1
                for g in range(2):  # each g covers 4 ctx chunks
                    vps = ps.tile([128, 1024], F32, tag="w", name=f"vps{qq}{g}")
                    for k in range(4):
                        ci = qq * 8 + g * 4 + k
                        for i in range(2):
                            nc.tensor.matmul(vps[:, k * 256:(k + 1) * 256],
                                             yslice(i, ci * 128, 128),
                                             wv_t[:, i * 256:(i + 1) * 256],
                                             start=(i == 0), stop=(i == 1))
                    dst = v3[:, qq * 8 + g * 4:qq * 8 + (g + 1) * 4, :]
                    if ev % 2 == 0:
                        nc.scalar.copy(dst, vps)
                    else:
                        nc.vector.tensor_copy(dst, vps)
                    ev += 1

            DR = mybir.MatmulPerfMode.DoubleRow
            NP2 = NBLK // 2  # 16 chunk pairs

            def attn_score(j):
                """S^T and exp for chunk pair j (one DoubleRow matmul per chunk)"""
                sps = ps.tile([128, 1024], F32, tag="w", name=f"sps{j}")
                for h in range(2):
                    i = 2 * j + h
                    nc.tensor.matmul(sps[:, h * 512:(h + 1) * 512],
                                     kt3[:, :, i * 128:(i + 1) * 128], qp3,
                                     start=True, stop=True, perf_mode=DR)
                pt = ptp.tile([128, 2, QS], FP8, tag="pt", name=f"pt{j}")
                nc.scalar.activation(pt, sps, AF.Exp)
                return pt

            def attn_accum(j, pt):
                """P@V and colsum for chunk pair j (DoubleRow over the pair)"""
                first, last = (j == 0), (j == NP2 - 1)
                for cb in range(2):
                    nc.tensor.matmul(
                        att2[:, cb * 512:(cb + 1) * 512],
                        v3[:, 2 * j:2 * j + 2, cb * 128:(cb + 1) * 128],
                        pt, start=first, stop=last, perf_mode=DR)
                nc.tensor.matmul(csum, ones3, pt, start=first, stop=last,
                                 perf_mode=DR)

            kv_quarter(0)
            p0 = attn_score(0)
            for j in range(1, NP2):
                if j % 4 == 0:
                    kv_quarter(j // 4)
                p1 = attn_score(j)
                attn_accum(j - 1, p0)
                p0 = p1
            attn_accum(NP2 - 1, p0)

            # evacuate the UN-normalized attention (both copies on the vector
            # engine) while the scalar engine computes 1/colsum via exp(-ln(x));
            # the softmax normalization is applied per-column after Wo instead
            # (z_q = Wo @ attT_q * (1/d_q) + bo commutes with the column scale).
            attnT2 = sb.tile([128, 1024], MMDT)
            nc.vector.tensor_copy(attnT2[:, 0:512], att2[:, 0:512])
            nc.vector.tensor_copy(attnT2[:, 512:1024], att2[:, 512:1024])
            lncs = sb.tile([1, QS], F32)
            nc.scalar.activation(lncs, csum[0:1, :], AF.Ln)
            rr = sb.tile([1, QS], MMDT)
            nc.scalar.activation(rr, lncs, AF.Exp, scale=-1.0)
            # keep the PE clock warm while the recip/evac chain runs
            warm(attnT2[:, 0:512], 4)

            # z = Wo @ attT_unnorm
            zps = ps.tile([128, 1024], F32, tag="att", bufs=1, name="zps")
            for cb in range(2):
                nc.tensor.matmul(zps[:, cb * 512:(cb + 1) * 512],
                                 wsl(wo_t, 0, cb), attnT2[:, 0:512],
                                 start=True, stop=False)
                nc.tensor.matmul(zps[:, cb * 512:(cb + 1) * 512],
                                 wsl(wo_t, 1, cb), attnT2[:, 512:1024],
                                 start=False, stop=True)
            rb = ps.tile([128, QS], F32, tag="w", name="rb")
            nc.tensor.matmul(rb, ones_r, rr, start=True, stop=True)
            rbs = sb.tile([128, QS], MMDT)
            nc.scalar.copy(rbs, rb)
            warm(attnT2[:, 0:512], 6)
            zs2 = sb.tile([128, 1024], MMDT)
            zt2m = sb.tile([128, 1024], MMDT)
            for cb in range(2):
                nc.vector.tensor_mul(zt2m[:, cb * 512:(cb + 1) * 512],
                                     zps[:, cb * 512:(cb + 1) * 512], rbs)
                nc.scalar.activation(zs2[:, cb * 512:(cb + 1) * 512],
                                     zt2m[:, cb * 512:(cb + 1) * 512], AF.Identity,
                                     bias=bvec[cb][:, 1:2])
            zsq2 = sb.tile([128, 1024], MMDT)
            nc.scalar.square(zsq2, zs2)

            szp = ps.tile([128, QS], F32, tag="csum", bufs=1, name="szp")
            nc.tensor.matmul(szp, ones_cb, zs2[:, 0:512], start=True, stop=False)
            nc.tensor.matmul(szp, ones_cb, zs2[:, 512:1024], start=False, stop=True)
            sqp = ps.tile([128, QS], F32, tag="w", name="sqp")
            nc.tensor.matmul(sqp, ones_cb, zsq2[:, 0:512], start=True, stop=False)
            nc.tensor.matmul(sqp, ones_cb, zsq2[:, 512:1024], start=False, stop=True)

            warm(zsq2[:, 0:512], 8)
            s2 = sb.tile([1, QS], F32)
            nc.scalar.square(s2, szp[0:1, :])
            var = sb.tile([1, QS], F32)
            nc.vector.scalar_tensor_tensor(var, sqp[0:1, :], float(C), s2,
                                           op0=ALU.mult, op1=ALU.subtract)
            lnv = sb.tile([1, QS], F32)
            nc.scalar.activation(lnv, var, AF.Ln, bias=eps2v)
            rstd = sb.tile([1, QS], MMDT)
            nc.scalar.activation(rstd, lnv, AF.Exp, scale=-0.5, bias=lnCv)
            neg_mean = sb.tile([1, QS], F32)
            nc.vector.tensor_scalar_mul(neg_mean, szp[0:1, :], -1.0 / C)
            nmrs = sb.tile([1, QS], MMDT)
            nc.vector.tensor_mul(nmrs, neg_mean, rstd)

            rstd_b = ps.tile([128, QS], F32, tag="w", name="rstd_b")
            nc.tensor.matmul(rstd_b, ones_r, rstd, start=True, stop=True)
            nmrs_b = ps.tile([128, QS], F32, tag="w", name="nmrs_b")
            nc.tensor.matmul(nmrs_b, ones_r, nmrs, start=True, stop=True)

            warm(zsq2[:, 512:1024], 6)
            zln2 = sb.tile([128, 1024], MMDT)
            for cb in range(2):
                zt = sb.tile([128, QS], MMDT, name=f"zt{cb}")
                nc.vector.tensor_mul(zt, zs2[:, cb * 512:(cb + 1) * 512], rstd_b)
                nc.vector.tensor_add(zln2[:, cb * 512:(cb + 1) * 512], zt, nmrs_b)

            # MLP + residual
            hs = [sb.tile([128, QS], MMDT, name=f"hs{i}") for i in range(4)]
            for hb in range(4):
                hps = ps.tile([128, QS], F32, tag="w", name=f"hps{hb}")
                nc.tensor.matmul(hps, wsl(w1_t, 0, hb), zln2[:, 0:512],
                                 start=True, stop=False)
                nc.tensor.matmul(hps, wsl(w1_t, 1, hb), zln2[:, 512:1024],
                                 start=False, stop=True)
                nc.scalar.activation(hs[hb], hps, AF.Gelu, bias=b1p[:, hb:hb + 1])

            tps2 = ps.tile([128, 1024], F32, tag="att", bufs=1, name="tps2")
            for cb in range(2):
                for hb in range(4):
                    nc.tensor.matmul(
                        tps2[:, cb * 512:(cb + 1) * 512],
                        w2_t[:, hb * 256 + cb * 128:hb * 256 + (cb + 1) * 128],
                        hs[hb], start=(hb == 0), stop=(hb == 3))
            ot = [sb.tile([128, QS], F32, name=f"ot{i}") for i in range(2)]
            for cb in range(2):
                nc.vector.scalar_tensor_tensor(
                    ot[cb], tps2[:, cb * 512:(cb + 1) * 512], bvec[cb][:, 2:3],
                    xf[cb], op0=ALU.add, op1=ALU.add)
            # spread the output store over four DMA queues
            nc.sync.dma_start(d_out[0:128, 0:256], ot[0][:, 0:256])
            nc.scalar.dma_start(d_out[0:128, 256:512], ot[0][:, 256:512])
            nc.gpsimd.dma_start(d_out[128:256, 0:256], ot[1][:, 0:256])
            nc.sync.dma_start(d_out[128:256, 256:512], ot[1][:, 256:512])

    nc.compile()
    return nc


_NC = None


def _get_nc():
    global _NC
    if _NC is None:
        _NC = _build_nc()
    return _NC


def _pack_rows(a, nchunk):
    """(nchunk*128, W) -> (128, nchunk*W) with row-chunks side by side."""
    w = a.shape[1]
    out = np.empty((128, nchunk * w), a.dtype)
    for i in range(nchunk):
        out[:, i * w:(i + 1) * w] = a[i * 128:(i + 1) * 128, :]
    return out


def prep_in_maps(x, y, Wq, bq, Wk, bk, Wv, bv, Wo, bo, ln_w, ln_b, W1, b1, W2, b2):
    f = lambda a: np.asarray(a, dtype=np.float32)
    x, y = f(x), f(y)
    Wq, bq, Wk, Wv, bv, Wo, bo = f(Wq), f(bq), f(Wk), f(Wv), f(bv), f(Wo), f(bo)
    ln_w, ln_b, W1, b1, W2, b2 = f(ln_w), f(ln_b), f(W1), f(b1), f(W2), f(b2)

    mmnp = mybir.dt.np(MMDT)
    g = lambda a: np.ascontiguousarray(a).astype(mmnp)

    x_cm = np.ascontiguousarray(x.reshape(C, HW))
    y_cm = np.ascontiguousarray(y.reshape(C, NCTX))

    # host-side algebraic folds
    bo_p = (Wo.astype(np.float64) @ bv.astype(np.float64) + bo).astype(np.float32)
    b1_p = (W1.astype(np.float64) @ ln_b.astype(np.float64) + b1).astype(np.float32)
    W1p = (W1 * ln_w[None, :]).astype(np.float32)

    bvec = np.stack([bq / 16.0, bo_p, b2], axis=1).astype(np.float32)  # (256,3)

    common = {
        "y_mm": g(_pack_rows(y_cm, 2)),
        "wq_mm": g(_pack_rows(Wq.T, 2)),
        "wk_mm": g(_pack_rows(Wk.T, 2)),
        "wv_mm": g(_pack_rows(Wv.T, 2)),
        "wo_mm": g(_pack_rows(Wo.T, 2)),
        "w1_mm": g(_pack_rows(W1p.T, 2)),
        "w2_mm": g(_pack_rows(W2.T, 4)),
        "bvec": bvec,
        "b1p": np.ascontiguousarray(b1_p.reshape(4, 128).T),
        "ones_c": np.ones((128, 256), mybir.dt.np(FP8)),
        "ones_cb": np.ones((128, 128), mmnp),
        "ones_r": np.ones((1, 128), mmnp),
    }
    in_maps = []
    for i in range(NCORES):
        m = dict(common)
        xs = np.ascontiguousarray(x_cm[:, i * QS:(i + 1) * QS])
        m["x_f32"] = xs
        m["x_mm"] = g(_pack_rows(xs, 2))
        in_maps.append(m)
    return in_maps


def kernel(**inputs):
    in_maps = prep_in_maps(**inputs)
    nc = _get_nc()
    res = bass_utils.run_bass_kernel_spmd(nc, in_maps, core_ids=list(range(NCORES)))
    t = np.concatenate([res.results[i]["out_sh"] for i in range(NCORES)], axis=1)
    return t.reshape(1, C, 64, 64)
